# revision 1
# baseline (speedup 1.0000x reference)
"""NNUE (HalfKA) forward kernel for Trainium2, data-parallel over batch on 8 cores.

v3: fp8(e4m3) embedding table [22528, 1280] (1024 ft + 8 psqt + 248 zero pad;
rows 1280B). Per 128-sample tile and perspective: 4 dma_gather calls of 1024
rows (8 features x 128 samples, feature-major). Reduce split: gather 0 on DVE
(bf16), gathers 1-3 via PE identity-matmul PSUM accumulate (exact); psqt cols
of the PE gathers via a tiny DVE tree. ft_bias added exactly in f32 after the
merge. MLP tail as v2 (host-prepped stm swap and bucket one-hot masks).
"""
import numpy as np
import ml_dtypes

import concourse.bacc as bacc
import concourse.bass as bass
import concourse.tile as tile
import concourse.mybir as mybir
from concourse.bass_utils import run_bass_kernel_spmd
from concourse.masks import make_identity
from concourse.library_config import mlp

F32 = mybir.dt.float32
BF16 = mybir.dt.bfloat16
FP8 = mybir.dt.float8e4
I16 = mybir.dt.int16

V = 22528          # ft table rows
FT = 1024          # ft embedding dim
PSQT = 8           # psqt buckets
D = FT + PSQT      # used row prefix (1032)
E = 1280           # gathered row length in fp8 bytes (256B-aligned)
B = 8192
FEATS = 32         # features per bag
NCORES = 8
BC = B // NCORES   # samples per core
P = 128            # partitions
T = BC // P        # sample tiles per core (8)
NI = 1024          # rows per dma_gather (8 features x 128 samples)
GPB = FEATS // 8   # gathers per bag per tile (4)
L2 = 15

_CACHE = {}


def _build_nc(reps=1, nq=2, gbufs=12, pe_g=4):
    nc = bacc.Bacc("TRN2", target_bir_lowering=False, debug=False,
                   num_swdge_queues=nq)

    tbl = nc.dram_tensor("tbl", [V, E], FP8, kind="ExternalInput")
    idx = nc.dram_tensor("idx", [P, T * 2 * GPB * (NI // 16)], I16,
                         kind="ExternalInput")
    m0 = nc.dram_tensor("m0", [P, T * 128], F32, kind="ExternalInput")
    m1 = nc.dram_tensor("m1", [P, T * 256], F32, kind="ExternalInput")
    m8 = nc.dram_tensor("m8", [P, T * 8], F32, kind="ExternalInput")
    cbias = nc.dram_tensor("cbias", [1, D], F32, kind="ExternalInput")
    w0T = nc.dram_tensor("w0T", [P, FT], F32, kind="ExternalInput")
    w1T = nc.dram_tensor("w1T", [32, 256], F32, kind="ExternalInput")
    w2T = nc.dram_tensor("w2T", [32, 8], F32, kind="ExternalInput")
    b0 = nc.dram_tensor("b0", [1, 128], F32, kind="ExternalInput")
    b1 = nc.dram_tensor("b1", [1, 256], F32, kind="ExternalInput")
    b2 = nc.dram_tensor("b2", [1, 8], F32, kind="ExternalInput")
    out = nc.dram_tensor("out", [BC, 1], F32, kind="ExternalOutput")

    with tile.TileContext(nc) as tc:
        with tc.tile_pool(name="const", bufs=1) as cp, \
             tc.tile_pool(name="gat", bufs=gbufs) as gpool, \
             tc.tile_pool(name="part", bufs=3) as ppool, \
             tc.tile_pool(name="accs", bufs=2) as apool, \
             tc.tile_pool(name="small", bufs=2) as spool, \
             tc.tile_pool(name="psum", bufs=2, space="PSUM") as pp, \
             tc.tile_pool(name="psred", bufs=2, space="PSUM") as pred, \
             tc.tile_pool(name="psacc", bufs=2, space="PSUM") as ppacc:

            nc.gpsimd.load_library(mlp)

            # ---- constants, loaded once ----
            ident = cp.tile([P, P], F32)
            make_identity(nc, ident[:])
            ident8 = cp.tile([P, P], FP8)
            nc.scalar.copy(ident8[:], ident[:])
            id2 = cp.tile([P, 2 * P], FP8)
            nc.scalar.copy(id2[:, 0:P], ident[:])
            nc.scalar.copy(id2[:, P:2 * P], ident[:])
            idx_sb = cp.tile([P, T * 2 * GPB * (NI // 16)], I16)
            nc.sync.dma_start(idx_sb[:], idx[:])
            cbias_sb = cp.tile([P, D], F32)
            nc.sync.dma_start(cbias_sb[:], cbias[:].to_broadcast((P, D)))
            w0T_sb = cp.tile([P, FT], F32)
            nc.sync.dma_start(w0T_sb[:], w0T[:])
            w1T_sb = cp.tile([32, 256], F32)
            nc.sync.dma_start(w1T_sb[:], w1T[:])
            w2T_sb = cp.tile([32, 8], F32)
            nc.sync.dma_start(w2T_sb[:], w2T[:])
            b0_sb = cp.tile([P, 128], F32)
            nc.sync.dma_start(b0_sb[:], b0[:].to_broadcast((P, 128)))
            b1_sb = cp.tile([P, 256], F32)
            nc.sync.dma_start(b1_sb[:], b1[:].to_broadcast((P, 256)))
            b2_sb = cp.tile([P, 8], F32)
            nc.sync.dma_start(b2_sb[:], b2[:].to_broadcast((P, 8)))
            # all-tile bucket one-hot masks, loaded once
            m0_all = cp.tile([P, T * 128], F32)
            nc.sync.dma_start(m0_all[:], m0[:])
            m1_all = cp.tile([P, T * 256], F32)
            nc.sync.dma_start(m1_all[:], m1[:])
            m8_all = cp.tile([P, T * 8], F32)
            nc.sync.dma_start(m8_all[:], m8[:])

            for t in [t for _ in range(reps) for t in range(T)]:
                rows = slice(t * P, (t + 1) * P)
                m0_t = m0_all[:, t * 128:(t + 1) * 128]
                m1_t = m1_all[:, t * 256:(t + 1) * 256]
                m8_t = m8_all[:, t * 8:(t + 1) * 8]

                # ---- gather + reduce both bags ----
                accs = []
                for bag in range(2):
                    acc = apool.tile([P, D], F32,
                                     tag="acc_w" if bag == 0 else "acc_b")
                    bufs = []
                    for g in range(GPB):
                        j = (t * 2 + bag) * GPB + g
                        col = j * (NI // 16)
                        buf = gpool.tile([P, 8, E], FP8, tag="gather")
                        nc.gpsimd.dma_gather(
                            buf[:], tbl[:], idx_sb[:, col:col + NI // 16],
                            NI, NI, E, transpose=False, queue_num=j % nq)
                        bufs.append(buf)

                    # DVE: gathers 0..nd-1 full 1032 cols (fp8 -> bf16 tree)
                    nd = GPB - pe_g
                    dvp = None
                    if nd > 0:
                        p4 = ppool.tile([P, 4, D], BF16, tag="p4")
                        nc.vector.tensor_add(p4[:], bufs[0][:, 0:4, 0:D],
                                             bufs[0][:, 4:8, 0:D])
                        if nd > 1:
                            tmp = ppool.tile([P, 4, D], BF16, tag="tmp")
                            for g in range(1, nd):
                                nc.vector.tensor_add(tmp[:], bufs[g][:, 0:4, 0:D],
                                                     bufs[g][:, 4:8, 0:D])
                                nc.vector.tensor_add(p4[:], p4[:], tmp[:])
                        q2 = ppool.tile([P, 2 * D], BF16, tag="q2")
                        nc.vector.tensor_add(
                            q2[:].rearrange("p (c d) -> p c d", c=2),
                            p4[:, 0:2, :], p4[:, 2:4, :])
                        dvp = ppool.tile([P, D], BF16, tag="dvp")
                        nc.vector.tensor_add(dvp[:], q2[:, 0:D], q2[:, D:2 * D])

                    if pe_g > 0:
                        # PE: last pe_g gathers, ft cols via identity matmuls
                        psA = pred.tile([P, 512], F32, tag="psA", space="PSUM")
                        psB = pred.tile([P, 512], F32, tag="psB", space="PSUM")
                        n_mm = pe_g * 4
                        i_mm = 0
                        i2v = id2[:].rearrange("p (two f) -> p two f", two=2)
                        DR = mybir.MatmulPerfMode.DoubleRow
                        for g in range(nd, GPB):
                            for flp in range(0, 8, 2):
                                st = i_mm == 0
                                sp = i_mm == n_mm - 1
                                nc.tensor.matmul(
                                    psA[:], lhsT=i2v,
                                    rhs=bufs[g][:, flp:flp + 2, 0:512],
                                    start=st, stop=sp, perf_mode=DR)
                                nc.tensor.matmul(
                                    psB[:], lhsT=i2v,
                                    rhs=bufs[g][:, flp:flp + 2, 512:1024],
                                    start=st, stop=sp, perf_mode=DR)
                                i_mm += 1

                        # psqt cols of PE gathers: small DVE tree on [P, 8*8]
                        pq = ppool.tile([P, 8 * PSQT], BF16, tag="pq")
                        pqv = pq[:].rearrange("p (c d) -> p c d", c=8)
                        if pe_g >= 2:
                            nc.vector.tensor_add(
                                pqv, bufs[nd][:, 0:8, FT:FT + PSQT],
                                bufs[nd + 1][:, 0:8, FT:FT + PSQT])
                            for g in range(nd + 2, GPB):
                                nc.vector.tensor_add(
                                    pqv, pqv, bufs[g][:, 0:8, FT:FT + PSQT])
                        else:
                            nc.vector.tensor_scalar_mul(
                                pqv, bufs[nd][:, 0:8, FT:FT + PSQT], 1.0)
                        nc.vector.tensor_add(pq[:, 0:32], pq[:, 0:32], pq[:, 32:64])
                        nc.vector.tensor_add(pq[:, 0:16], pq[:, 0:16], pq[:, 16:32])
                        nc.vector.tensor_add(pq[:, 0:8], pq[:, 0:8], pq[:, 8:16])

                        # merge + exact f32 bias
                        if dvp is not None:
                            nc.vector.tensor_add(acc[:, 0:512], psA[:],
                                                 dvp[:, 0:512])
                            nc.vector.tensor_add(acc[:, 512:1024], psB[:],
                                                 dvp[:, 512:1024])
                            nc.vector.tensor_add(acc[:, FT:D], pq[:, 0:8],
                                                 dvp[:, FT:D])
                            nc.vector.tensor_add(acc[:], acc[:], cbias_sb[:])
                        else:
                            nc.vector.tensor_add(acc[:, 0:512], psA[:],
                                                 cbias_sb[:, 0:512])
                            nc.vector.tensor_add(acc[:, 512:1024], psB[:],
                                                 cbias_sb[:, 512:1024])
                            nc.vector.tensor_add(acc[:, FT:D], pq[:, 0:8],
                                                 cbias_sb[:, FT:D])
                    else:
                        nc.vector.tensor_add(acc[:], dvp[:], cbias_sb[:])
                    accs.append(acc)
                acc_stm, acc_opp = accs

                # ---- psqt: (stm-side - opp-side) bucket column, / 32 ----
                pdm = spool.tile([P, 8], F32, tag="pdm")
                nc.vector.tensor_sub(pdm[:], acc_stm[:, FT:D], acc_opp[:, FT:D])
                nc.vector.tensor_mul(pdm[:], pdm[:], m8_t)
                pd2 = spool.tile([P, 4], F32, tag="pd2")
                nc.vector.tensor_add(pd2[:], pdm[:, 0:4], pdm[:, 4:8])
                nc.vector.tensor_add(pd2[:, 0:2], pd2[:, 0:2], pd2[:, 2:4])
                psel = spool.tile([P, 1], F32, tag="psel")
                nc.vector.tensor_add(psel[:], pd2[:, 0:1], pd2[:, 1:2])
                nc.vector.tensor_scalar_mul(psel[:], psel[:], 1.0 / 32.0)

                # ---- pairwise: ft halves written in place ----
                H = FT // 2
                ft_halves = []
                for acc in (acc_stm, acc_opp):
                    nc.vector.tensor_scalar(
                        out=acc[:, 0:H], in0=acc[:, 0:H],
                        scalar1=0.0, scalar2=127.0,
                        op0=mybir.AluOpType.max, op1=mybir.AluOpType.min)
                    nc.vector.tensor_scalar(
                        out=acc[:, H:FT], in0=acc[:, H:FT],
                        scalar1=0.0, scalar2=127.0,
                        op0=mybir.AluOpType.max, op1=mybir.AluOpType.min)
                    nc.vector.tensor_mul(acc[:, 0:H], acc[:, 0:H], acc[:, H:FT])
                    ft_halves.append(acc[:, 0:H])

                # ---- fc0: transpose ft tiles, matmul all stacks ----
                mm = ppacc.tile([P, 256], F32, tag="mm", space="PSUM")
                o0p = mm[:, 0:128]
                for k in range(8):
                    col = (k % 4) * P
                    tp = pp.tile([P, P], F32, tag="tpose", space="PSUM")
                    nc.tensor.transpose(tp[:], ft_halves[k // 4][:, col:col + P], ident[:])
                    ftT = spool.tile([P, P], F32, tag="ftT")
                    nc.scalar.copy(ftT[:], tp[:])
                    nc.tensor.matmul(
                        o0p, lhsT=ftT[:], rhs=w0T_sb[:, k * P:(k + 1) * P],
                        start=(k == 0), stop=(k == 7))

                # ---- fc0 bias + mask select ----
                o0m = spool.tile([P, 128], F32, tag="o0m")
                nc.vector.tensor_add(o0m[:], o0p, b0_sb[:])
                nc.vector.tensor_mul(o0m[:], o0m[:], m0_t)
                o0h = spool.tile([P, 64], F32, tag="o0h")
                nc.vector.tensor_add(o0h[:], o0m[:, 0:64], o0m[:, 64:128])
                nc.vector.tensor_add(o0h[:, 0:32], o0h[:, 0:32], o0h[:, 32:64])
                o0 = spool.tile([P, 16], F32, tag="o0")
                nc.vector.tensor_add(o0[:], o0h[:, 0:16], o0h[:, 16:32])

                # ---- slab activations ----
                slab = spool.tile([P, 32], F32, tag="slab")
                nc.vector.memset(slab[:, 30:32], 0.0)
                sq = spool.tile([P, L2], F32, tag="sq")
                nc.vector.tensor_mul(sq[:], o0[:, 0:L2], o0[:, 0:L2])
                nc.vector.tensor_scalar(
                    out=slab[:, 0:L2], in0=sq[:],
                    scalar1=1.0 / 524288.0, scalar2=127.0,
                    op0=mybir.AluOpType.mult, op1=mybir.AluOpType.min)
                nc.vector.tensor_scalar(
                    out=slab[:, L2:2 * L2], in0=o0[:, 0:L2],
                    scalar1=1.0 / 64.0, scalar2=0.0,
                    op0=mybir.AluOpType.mult, op1=mybir.AluOpType.max)
                nc.vector.tensor_scalar_min(slab[:, L2:2 * L2], slab[:, L2:2 * L2], 127.0)

                # ---- fc1 ----
                tpf = pp.tile([P, P], F32, tag="tpose", space="PSUM")
                tps = tpf[0:32, :]
                nc.tensor.transpose(tps, slab[:], ident[:])
                slabT = spool.tile([32, P], F32, tag="slabT")
                nc.scalar.copy(slabT[:], tps)
                o1p = mm[:, 0:256]
                nc.tensor.matmul(o1p, lhsT=slabT[:], rhs=w1T_sb[:], start=True, stop=True)
                o1m = spool.tile([P, 256], F32, tag="o1m")
                nc.vector.tensor_add(o1m[:], o1p, b1_sb[:])
                nc.vector.tensor_mul(o1m[:], o1m[:], m1_t)
                o1h = spool.tile([P, 128], F32, tag="o1h")
                nc.vector.tensor_add(o1h[:], o1m[:, 0:128], o1m[:, 128:256])
                nc.vector.tensor_add(o1h[:, 0:64], o1h[:, 0:64], o1h[:, 64:128])
                o1 = spool.tile([P, 32], F32, tag="o1")
                nc.vector.tensor_add(o1[:], o1h[:, 0:32], o1h[:, 32:64])
                nc.vector.tensor_scalar(
                    out=o1[:], in0=o1[:],
                    scalar1=1.0 / 64.0, scalar2=0.0,
                    op0=mybir.AluOpType.mult, op1=mybir.AluOpType.max)
                nc.vector.tensor_scalar_min(o1[:], o1[:], 127.0)

                # ---- fc2 ----
                tpg = pp.tile([P, P], F32, tag="tpose", space="PSUM")
                tpa = tpg[0:32, :]
                nc.tensor.transpose(tpa, o1[:], ident[:])
                ac1T = spool.tile([32, P], F32, tag="ac1T")
                nc.scalar.copy(ac1T[:], tpa)
                o2p = mm[:, 0:8]
                nc.tensor.matmul(o2p, lhsT=ac1T[:], rhs=w2T_sb[:], start=True, stop=True)
                o2m = spool.tile([P, 8], F32, tag="o2m")
                nc.vector.tensor_add(o2m[:], o2p, b2_sb[:])
                nc.vector.tensor_mul(o2m[:], o2m[:], m8_t)
                o2h = spool.tile([P, 4], F32, tag="o2h")
                nc.vector.tensor_add(o2h[:], o2m[:, 0:4], o2m[:, 4:8])
                nc.vector.tensor_add(o2h[:, 0:2], o2h[:, 0:2], o2h[:, 2:4])
                res = spool.tile([P, 1], F32, tag="res")
                nc.vector.tensor_add(res[:], o2h[:, 0:1], o2h[:, 1:2])

                # ---- skip + psqt + output ----
                skip = spool.tile([P, 1], F32, tag="skip")
                nc.vector.tensor_scalar_mul(skip[:], o0[:, L2:16], 9600.0 / 8128.0 / 16.0)
                nc.vector.tensor_add(res[:], res[:], skip[:])
                nc.vector.tensor_add(res[:], res[:], psel[:])
                nc.sync.dma_start(out[rows, :], res[:])

    nc.compile()
    return nc


def _prep_inputs(inputs):
    """Host-side prep: fp8 table, wrapped int16 gather indices (stm-swapped),
    transposed/prescaled weights and bucket one-hot masks."""
    ft_w = np.asarray(inputs["ft_w"], dtype=np.float32)
    psqt_w = np.asarray(inputs["psqt_w"], dtype=np.float32)
    ft_bias = np.asarray(inputs["ft_bias"], dtype=np.float32)
    tbl = np.zeros((V, E), dtype=ml_dtypes.float8_e4m3fn)
    tbl[:, :FT] = ft_w.astype(ml_dtypes.float8_e4m3fn)
    tbl[:, FT:D] = psqt_w.astype(ml_dtypes.float8_e4m3fn)
    cbias = np.concatenate([ft_bias, np.zeros(PSQT, np.float32)]).reshape(1, D)

    fc0_w = np.asarray(inputs["fc0_w"], dtype=np.float32)  # [8,16,1024]
    fc1_w = np.asarray(inputs["fc1_w"], dtype=np.float32)  # [8,32,32]
    fc2_w = np.asarray(inputs["fc2_w"], dtype=np.float32)  # [8,1,32]
    a = fc0_w.transpose(2, 0, 1).reshape(FT, 128) * (1.0 / 128.0)   # [h, (s,j)]
    w0T = np.ascontiguousarray(
        a.reshape(8, 128, 128).transpose(1, 0, 2).reshape(128, FT))
    w1T = np.ascontiguousarray(fc1_w.transpose(2, 0, 1).reshape(32, 256))
    w2T = np.ascontiguousarray(fc2_w[:, 0, :].T * (1.0 / 16.0))     # [32, 8]
    b0 = np.asarray(inputs["fc0_b"], np.float32).reshape(1, 128)
    b1 = np.asarray(inputs["fc1_b"], np.float32).reshape(1, 256)
    b2 = np.asarray(inputs["fc2_b"], np.float32).reshape(1, 8) * (1.0 / 16.0)

    w_feats = np.asarray(inputs["w_feats"]).astype(np.int16)
    b_feats = np.asarray(inputs["b_feats"]).astype(np.int16)
    stm = np.asarray(inputs["stm"]).astype(np.float32)
    bucket = np.asarray(inputs["bucket"]).astype(np.int64)

    in_maps = []
    for c in range(NCORES):
        s = slice(c * BC, (c + 1) * BC)
        wf = w_feats[c * BC * FEATS:(c + 1) * BC * FEATS].reshape(T, P, FEATS)
        bf = b_feats[c * BC * FEATS:(c + 1) * BC * FEATS].reshape(T, P, FEATS)
        st_tp = stm[s].reshape(T, P, 1) > 0.5
        f_stm = np.where(st_tp, bf, wf)
        f_opp = np.where(st_tp, wf, bf)
        # gather j=(t*2+bag)*GPB+g covers features 8g..8g+7, feature-major:
        # local id i = f_local*128 + sample
        blocks = np.empty((T, 2, GPB, NI), np.int16)
        for bag, arr in enumerate((f_stm, f_opp)):
            a8 = arr.reshape(T, P, GPB, 8).transpose(0, 2, 3, 1)  # [T,GPB,8,P]
            blocks[:, bag, :, :] = a8.reshape(T, GPB, NI)
        flat = blocks.reshape(T * 2 * GPB, NI)
        wrapped = flat.reshape(-1, NI // 16, 16).transpose(0, 2, 1)
        idx_arr = np.zeros((P, T * 2 * GPB * (NI // 16)), np.int16)
        cols = wrapped.transpose(1, 0, 2).reshape(16, -1)
        for g in range(8):
            idx_arr[g * 16:(g + 1) * 16, :] = cols

        bk = bucket[s]
        m0 = (bk[:, None] == (np.arange(128) // 16)).astype(np.float32)
        m1 = (bk[:, None] == (np.arange(256) // 32)).astype(np.float32)
        m8 = (bk[:, None] == np.arange(8)).astype(np.float32)
        # SBUF layout: partition = sample-in-tile, cols = tile-major blocks
        m0 = np.ascontiguousarray(
            m0.reshape(T, P, 128).transpose(1, 0, 2).reshape(P, T * 128))
        m1 = np.ascontiguousarray(
            m1.reshape(T, P, 256).transpose(1, 0, 2).reshape(P, T * 256))
        m8 = np.ascontiguousarray(
            m8.reshape(T, P, 8).transpose(1, 0, 2).reshape(P, T * 8))
        in_maps.append({
            "tbl": tbl, "idx": idx_arr, "cbias": cbias,
            "m0": m0, "m1": m1, "m8": m8,
            "w0T": w0T, "w1T": w1T, "w2T": w2T,
            "b0": b0, "b1": b1, "b2": b2,
        })
    return in_maps


def kernel(**inputs) -> np.ndarray:
    if "nc" not in _CACHE:
        _CACHE["nc"] = _build_nc()
    nc = _CACHE["nc"]
    in_maps = _prep_inputs(inputs)
    r = run_bass_kernel_spmd(nc, in_maps, core_ids=list(range(NCORES)))
    return np.concatenate([r.results[c]["out"][:, 0] for c in range(NCORES)])



# revision 4
# speedup vs baseline: 240.5168x; 240.5168x over previous
"""NNUE (HalfKA) forward kernel for Trainium2, data-parallel over batch on 8 cores.

v4: fp8(e4m3) embedding table [22528, 1024] (ft columns only; 1024B rows).
The psqt term (8 cols, tiny) is computed exactly on the host and passed in as
a per-sample scalar, cutting gather traffic 20% vs v3's 1280B rows. Per
128-sample tile and perspective: 2 dma_gather calls of 2048 rows (16 features
x 128 samples, feature-major; half the calls of v3). Reduce entirely on PE:
identity-matmul DoubleRow PSUM accumulate (exact). ft_bias added in f32 after
the merge. MLP tail as v3 (host-prepped stm swap and bucket one-hot masks).
"""
import numpy as np
import ml_dtypes

import concourse.bacc as bacc
import concourse.bass as bass
import concourse.tile as tile
import concourse.mybir as mybir
from concourse.bass_utils import run_bass_kernel_spmd
from concourse.masks import make_identity
from concourse.library_config import mlp

F32 = mybir.dt.float32
BF16 = mybir.dt.bfloat16
FP8 = mybir.dt.float8e4
I16 = mybir.dt.int16

V = 22528          # ft table rows
FT = 1024          # ft embedding dim
PSQT = 8           # psqt buckets (host-side now)
E = 1024           # gathered row length in fp8 bytes (256B-aligned)
B = 8192
FEATS = 32         # features per bag
NCORES = 8
BC = B // NCORES   # samples per core
P = 128            # partitions
T = BC // P        # sample tiles per core (8)
NI = 1024          # rows per dma_gather (FPG features x 128 samples)
FPG = NI // P      # features per gather (8)
GPB = FEATS // FPG  # gathers per bag per tile (4)
L2 = 15

_CACHE = {}


def _build_nc(reps=1, nq=2, gbufs=10, pe_g=None):
    # SWDGE descriptor ring must hold one full gather (16B per descriptor)
    scratch = max(16384, NI * 16)
    nc = bacc.Bacc("TRN2", target_bir_lowering=False, debug=False,
                   num_swdge_queues=nq, dynamic_dma_scratch_size=scratch)

    tbl = nc.dram_tensor("tbl", [V, E], FP8, kind="ExternalInput")
    idx = nc.dram_tensor("idx", [P, T * 2 * GPB * (NI // 16)], I16,
                         kind="ExternalInput")
    m0 = nc.dram_tensor("m0", [P, T * 128], F32, kind="ExternalInput")
    m1 = nc.dram_tensor("m1", [P, T * 256], F32, kind="ExternalInput")
    m8 = nc.dram_tensor("m8", [P, T * 8], F32, kind="ExternalInput")
    psel_d = nc.dram_tensor("psel", [P, T], F32, kind="ExternalInput")
    cbias = nc.dram_tensor("cbias", [1, FT], F32, kind="ExternalInput")
    w0T = nc.dram_tensor("w0T", [P, FT], F32, kind="ExternalInput")
    w1T = nc.dram_tensor("w1T", [32, 256], F32, kind="ExternalInput")
    w2T = nc.dram_tensor("w2T", [32, 8], F32, kind="ExternalInput")
    b0 = nc.dram_tensor("b0", [1, 128], F32, kind="ExternalInput")
    b1 = nc.dram_tensor("b1", [1, 256], F32, kind="ExternalInput")
    b2 = nc.dram_tensor("b2", [1, 8], F32, kind="ExternalInput")
    out = nc.dram_tensor("out", [BC, 1], F32, kind="ExternalOutput")

    with tile.TileContext(nc) as tc:
        with tc.tile_pool(name="const", bufs=1) as cp, \
             tc.tile_pool(name="gat", bufs=gbufs) as gpool, \
             tc.tile_pool(name="accs", bufs=2) as apool, \
             tc.tile_pool(name="small", bufs=2) as spool, \
             tc.tile_pool(name="psum", bufs=2, space="PSUM") as pp, \
             tc.tile_pool(name="psred", bufs=2, space="PSUM") as pred, \
             tc.tile_pool(name="psacc", bufs=2, space="PSUM") as ppacc:

            nc.gpsimd.load_library(mlp)

            # ---- constants, loaded once ----
            idx_sb = cp.tile([P, T * 2 * GPB * (NI // 16)], I16)
            nc.sync.dma_start(idx_sb[:], idx[:])
            ident = cp.tile([P, P], F32)
            make_identity(nc, ident[:])
            id2 = cp.tile([P, 2 * P], FP8)
            nc.scalar.copy(id2[:, 0:P], ident[:])
            nc.scalar.copy(id2[:, P:2 * P], ident[:])
            cbias_sb = cp.tile([P, FT], F32)
            nc.sync.dma_start(cbias_sb[:], cbias[:].to_broadcast((P, FT)))
            w0T_sb = cp.tile([P, FT], F32)
            nc.sync.dma_start(w0T_sb[:], w0T[:])
            w1T_sb = cp.tile([32, 256], F32)
            nc.sync.dma_start(w1T_sb[:], w1T[:])
            w2T_sb = cp.tile([32, 8], F32)
            nc.sync.dma_start(w2T_sb[:], w2T[:])
            b0_sb = cp.tile([P, 128], F32)
            nc.sync.dma_start(b0_sb[:], b0[:].to_broadcast((P, 128)))
            b1_sb = cp.tile([P, 256], F32)
            nc.sync.dma_start(b1_sb[:], b1[:].to_broadcast((P, 256)))
            b2_sb = cp.tile([P, 8], F32)
            nc.sync.dma_start(b2_sb[:], b2[:].to_broadcast((P, 8)))
            # all-tile bucket one-hot masks + host-side psqt, loaded once
            m0_all = cp.tile([P, T * 128], F32)
            nc.sync.dma_start(m0_all[:], m0[:])
            m1_all = cp.tile([P, T * 256], F32)
            nc.sync.dma_start(m1_all[:], m1[:])
            m8_all = cp.tile([P, T * 8], F32)
            nc.sync.dma_start(m8_all[:], m8[:])
            psel_all = cp.tile([P, T], F32)
            nc.sync.dma_start(psel_all[:], psel_d[:])

            for t in [t for _ in range(reps) for t in range(T)]:
                rows = slice(t * P, (t + 1) * P)
                m0_t = m0_all[:, t * 128:(t + 1) * 128]
                m1_t = m1_all[:, t * 256:(t + 1) * 256]
                m8_t = m8_all[:, t * 8:(t + 1) * 8]

                # ---- gather + reduce both bags ----
                accs = []
                for bag in range(2):
                    acc = apool.tile([P, FT], F32,
                                     tag="acc_w" if bag == 0 else "acc_b")
                    bufs = []
                    for g in range(GPB):
                        j = (t * 2 + bag) * GPB + g
                        col = j * (NI // 16)
                        buf = gpool.tile([P, FPG, E], FP8, tag="gather")
                        nc.gpsimd.dma_gather(
                            buf[:], tbl[:], idx_sb[:, col:col + NI // 16],
                            NI, NI, E, transpose=False, queue_num=j % nq)
                        bufs.append(buf)

                    # PE: all gathers reduced via identity matmuls (fp8,
                    # DoubleRow), accumulated exactly in f32 PSUM
                    psA = pred.tile([P, 512], F32, tag="psA", space="PSUM")
                    psB = pred.tile([P, 512], F32, tag="psB", space="PSUM")
                    n_mm = GPB * FPG // 2
                    i_mm = 0
                    i2v = id2[:].rearrange("p (two f) -> p two f", two=2)
                    DR = mybir.MatmulPerfMode.DoubleRow
                    for g in range(GPB):
                        for flp in range(0, FPG, 2):
                            st = i_mm == 0
                            sp = i_mm == n_mm - 1
                            nc.tensor.matmul(
                                psA[:], lhsT=i2v,
                                rhs=bufs[g][:, flp:flp + 2, 0:512],
                                start=st, stop=sp, perf_mode=DR)
                            nc.tensor.matmul(
                                psB[:], lhsT=i2v,
                                rhs=bufs[g][:, flp:flp + 2, 512:1024],
                                start=st, stop=sp, perf_mode=DR)
                            i_mm += 1

                    # merge + exact f32 bias
                    nc.vector.tensor_add(acc[:, 0:512], psA[:],
                                         cbias_sb[:, 0:512])
                    nc.vector.tensor_add(acc[:, 512:1024], psB[:],
                                         cbias_sb[:, 512:1024])
                    accs.append(acc)
                acc_stm, acc_opp = accs

                # ---- pairwise: ft halves written in place ----
                H = FT // 2
                ft_halves = []
                for acc in (acc_stm, acc_opp):
                    nc.vector.tensor_scalar(
                        out=acc[:, 0:H], in0=acc[:, 0:H],
                        scalar1=0.0, scalar2=127.0,
                        op0=mybir.AluOpType.max, op1=mybir.AluOpType.min)
                    nc.vector.tensor_scalar(
                        out=acc[:, H:FT], in0=acc[:, H:FT],
                        scalar1=0.0, scalar2=127.0,
                        op0=mybir.AluOpType.max, op1=mybir.AluOpType.min)
                    nc.vector.tensor_mul(acc[:, 0:H], acc[:, 0:H], acc[:, H:FT])
                    ft_halves.append(acc[:, 0:H])

                # ---- fc0: transpose ft tiles, matmul all stacks ----
                mm = ppacc.tile([P, 256], F32, tag="mm", space="PSUM")
                o0p = mm[:, 0:128]
                for k in range(8):
                    col = (k % 4) * P
                    tp = pp.tile([P, P], F32, tag="tpose", space="PSUM")
                    nc.tensor.transpose(tp[:], ft_halves[k // 4][:, col:col + P], ident[:])
                    ftT = spool.tile([P, P], F32, tag="ftT")
                    nc.scalar.copy(ftT[:], tp[:])
                    nc.tensor.matmul(
                        o0p, lhsT=ftT[:], rhs=w0T_sb[:, k * P:(k + 1) * P],
                        start=(k == 0), stop=(k == 7))

                # ---- fc0 bias + mask select ----
                o0m = spool.tile([P, 128], F32, tag="o0m")
                nc.vector.tensor_add(o0m[:], o0p, b0_sb[:])
                nc.vector.tensor_mul(o0m[:], o0m[:], m0_t)
                o0h = spool.tile([P, 64], F32, tag="o0h")
                nc.vector.tensor_add(o0h[:], o0m[:, 0:64], o0m[:, 64:128])
                nc.vector.tensor_add(o0h[:, 0:32], o0h[:, 0:32], o0h[:, 32:64])
                o0 = spool.tile([P, 16], F32, tag="o0")
                nc.vector.tensor_add(o0[:], o0h[:, 0:16], o0h[:, 16:32])

                # ---- slab activations ----
                slab = spool.tile([P, 32], F32, tag="slab")
                nc.vector.memset(slab[:, 30:32], 0.0)
                sq = spool.tile([P, L2], F32, tag="sq")
                nc.vector.tensor_mul(sq[:], o0[:, 0:L2], o0[:, 0:L2])
                nc.vector.tensor_scalar(
                    out=slab[:, 0:L2], in0=sq[:],
                    scalar1=1.0 / 524288.0, scalar2=127.0,
                    op0=mybir.AluOpType.mult, op1=mybir.AluOpType.min)
                nc.vector.tensor_scalar(
                    out=slab[:, L2:2 * L2], in0=o0[:, 0:L2],
                    scalar1=1.0 / 64.0, scalar2=0.0,
                    op0=mybir.AluOpType.mult, op1=mybir.AluOpType.max)
                nc.vector.tensor_scalar_min(slab[:, L2:2 * L2], slab[:, L2:2 * L2], 127.0)

                # ---- fc1 ----
                tpf = pp.tile([P, P], F32, tag="tpose", space="PSUM")
                tps = tpf[0:32, :]
                nc.tensor.transpose(tps, slab[:], ident[:])
                slabT = spool.tile([32, P], F32, tag="slabT")
                nc.scalar.copy(slabT[:], tps)
                o1p = mm[:, 0:256]
                nc.tensor.matmul(o1p, lhsT=slabT[:], rhs=w1T_sb[:], start=True, stop=True)
                o1m = spool.tile([P, 256], F32, tag="o1m")
                nc.vector.tensor_add(o1m[:], o1p, b1_sb[:])
                nc.vector.tensor_mul(o1m[:], o1m[:], m1_t)
                o1h = spool.tile([P, 128], F32, tag="o1h")
                nc.vector.tensor_add(o1h[:], o1m[:, 0:128], o1m[:, 128:256])
                nc.vector.tensor_add(o1h[:, 0:64], o1h[:, 0:64], o1h[:, 64:128])
                o1 = spool.tile([P, 32], F32, tag="o1")
                nc.vector.tensor_add(o1[:], o1h[:, 0:32], o1h[:, 32:64])
                nc.vector.tensor_scalar(
                    out=o1[:], in0=o1[:],
                    scalar1=1.0 / 64.0, scalar2=0.0,
                    op0=mybir.AluOpType.mult, op1=mybir.AluOpType.max)
                nc.vector.tensor_scalar_min(o1[:], o1[:], 127.0)

                # ---- fc2 ----
                tpg = pp.tile([P, P], F32, tag="tpose", space="PSUM")
                tpa = tpg[0:32, :]
                nc.tensor.transpose(tpa, o1[:], ident[:])
                ac1T = spool.tile([32, P], F32, tag="ac1T")
                nc.scalar.copy(ac1T[:], tpa)
                o2p = mm[:, 0:8]
                nc.tensor.matmul(o2p, lhsT=ac1T[:], rhs=w2T_sb[:], start=True, stop=True)
                o2m = spool.tile([P, 8], F32, tag="o2m")
                nc.vector.tensor_add(o2m[:], o2p, b2_sb[:])
                nc.vector.tensor_mul(o2m[:], o2m[:], m8_t)
                o2h = spool.tile([P, 4], F32, tag="o2h")
                nc.vector.tensor_add(o2h[:], o2m[:, 0:4], o2m[:, 4:8])
                nc.vector.tensor_add(o2h[:, 0:2], o2h[:, 0:2], o2h[:, 2:4])
                res = spool.tile([P, 1], F32, tag="res")
                nc.vector.tensor_add(res[:], o2h[:, 0:1], o2h[:, 1:2])

                # ---- skip + host-side psqt + output ----
                skip = spool.tile([P, 1], F32, tag="skip")
                nc.vector.tensor_scalar_mul(skip[:], o0[:, L2:16], 9600.0 / 8128.0 / 16.0)
                nc.vector.tensor_add(res[:], res[:], skip[:])
                nc.vector.tensor_add(res[:], res[:], psel_all[:, t:t + 1])
                nc.sync.dma_start(out[rows, :], res[:])

    nc.compile()
    return nc


def _prep_inputs(inputs):
    """Host-side prep: fp8 table, wrapped int16 gather indices (stm-swapped),
    transposed/prescaled weights, bucket one-hot masks, and the exact psqt
    term (tiny [V,8] table; computed here so the device gathers 1024B rows)."""
    ft_w = np.asarray(inputs["ft_w"], dtype=np.float32)
    psqt_w = np.asarray(inputs["psqt_w"], dtype=np.float32)
    ft_bias = np.asarray(inputs["ft_bias"], dtype=np.float32)
    tbl = ft_w.astype(ml_dtypes.float8_e4m3fn)
    cbias = ft_bias.reshape(1, FT)

    fc0_w = np.asarray(inputs["fc0_w"], dtype=np.float32)  # [8,16,1024]
    fc1_w = np.asarray(inputs["fc1_w"], dtype=np.float32)  # [8,32,32]
    fc2_w = np.asarray(inputs["fc2_w"], dtype=np.float32)  # [8,1,32]
    a = fc0_w.transpose(2, 0, 1).reshape(FT, 128) * (1.0 / 128.0)   # [h, (s,j)]
    w0T = np.ascontiguousarray(
        a.reshape(8, 128, 128).transpose(1, 0, 2).reshape(128, FT))
    w1T = np.ascontiguousarray(fc1_w.transpose(2, 0, 1).reshape(32, 256))
    w2T = np.ascontiguousarray(fc2_w[:, 0, :].T * (1.0 / 16.0))     # [32, 8]
    b0 = np.asarray(inputs["fc0_b"], np.float32).reshape(1, 128)
    b1 = np.asarray(inputs["fc1_b"], np.float32).reshape(1, 256)
    b2 = np.asarray(inputs["fc2_b"], np.float32).reshape(1, 8) * (1.0 / 16.0)

    w_feats = np.asarray(inputs["w_feats"]).astype(np.int64)
    b_feats = np.asarray(inputs["b_feats"]).astype(np.int64)
    stm = np.asarray(inputs["stm"]).astype(np.float32)
    bucket = np.asarray(inputs["bucket"]).astype(np.int64)

    # exact host-side psqt: (psqt_stm - psqt_opp)[b, bucket] / 32
    # (the /2 from the reference plus the final /16 folded together)
    psqt_wb = psqt_w[w_feats.reshape(B, FEATS)].sum(axis=1)  # [B, 8]
    psqt_bb = psqt_w[b_feats.reshape(B, FEATS)].sum(axis=1)
    stm_col = stm.reshape(B, 1)
    psqt_stm = psqt_wb * (1 - stm_col) + psqt_bb * stm_col
    psqt_opp = psqt_bb * (1 - stm_col) + psqt_wb * stm_col
    bidx = np.arange(B)
    psel = (psqt_stm[bidx, bucket] - psqt_opp[bidx, bucket]) / 32.0  # [B]

    w_feats16 = w_feats.astype(np.int16)
    b_feats16 = b_feats.astype(np.int16)

    in_maps = []
    for c in range(NCORES):
        s = slice(c * BC, (c + 1) * BC)
        wf = w_feats16[c * BC * FEATS:(c + 1) * BC * FEATS].reshape(T, P, FEATS)
        bf = b_feats16[c * BC * FEATS:(c + 1) * BC * FEATS].reshape(T, P, FEATS)
        st_tp = stm[s].reshape(T, P, 1) > 0.5
        f_stm = np.where(st_tp, bf, wf)
        f_opp = np.where(st_tp, wf, bf)
        # gather j=(t*2+bag)*GPB+g covers features FPG*g..FPG*(g+1)-1, feature-major:
        # local id i = f_local*128 + sample
        blocks = np.empty((T, 2, GPB, NI), np.int16)
        for bag, arr in enumerate((f_stm, f_opp)):
            ak = arr.reshape(T, P, GPB, FPG).transpose(0, 2, 3, 1)  # [T,GPB,FPG,P]
            blocks[:, bag, :, :] = ak.reshape(T, GPB, NI)
        flat = blocks.reshape(T * 2 * GPB, NI)
        wrapped = flat.reshape(-1, NI // 16, 16).transpose(0, 2, 1)
        idx_arr = np.zeros((P, T * 2 * GPB * (NI // 16)), np.int16)
        cols = wrapped.transpose(1, 0, 2).reshape(16, -1)
        for g in range(8):
            idx_arr[g * 16:(g + 1) * 16, :] = cols

        bk = bucket[s]
        m0 = (bk[:, None] == (np.arange(128) // 16)).astype(np.float32)
        m1 = (bk[:, None] == (np.arange(256) // 32)).astype(np.float32)
        m8 = (bk[:, None] == np.arange(8)).astype(np.float32)
        # SBUF layout: partition = sample-in-tile, cols = tile-major blocks
        m0 = np.ascontiguousarray(
            m0.reshape(T, P, 128).transpose(1, 0, 2).reshape(P, T * 128))
        m1 = np.ascontiguousarray(
            m1.reshape(T, P, 256).transpose(1, 0, 2).reshape(P, T * 256))
        m8 = np.ascontiguousarray(
            m8.reshape(T, P, 8).transpose(1, 0, 2).reshape(P, T * 8))
        psel_c = np.ascontiguousarray(
            psel[s].astype(np.float32).reshape(T, P).T)  # [P, T]
        in_maps.append({
            "tbl": tbl, "idx": idx_arr, "cbias": cbias,
            "m0": m0, "m1": m1, "m8": m8, "psel": psel_c,
            "w0T": w0T, "w1T": w1T, "w2T": w2T,
            "b0": b0, "b1": b1, "b2": b2,
        })
    return in_maps


def kernel(**inputs) -> np.ndarray:
    if "nc" not in _CACHE:
        _CACHE["nc"] = _build_nc()
    nc = _CACHE["nc"]
    in_maps = _prep_inputs(inputs)
    r = run_bass_kernel_spmd(nc, in_maps, core_ids=list(range(NCORES)))
    return np.concatenate([r.results[c]["out"][:, 0] for c in range(NCORES)])


# revision 7
# speedup vs baseline: 303.8915x; 1.2635x over previous
"""NNUE (HalfKA) forward kernel for Trainium2, data-parallel over batch on 8 cores.

v5: fp8(e4m3) embedding table, rows hold the 1024 ft columns at a 1280B
non-pow2 row stride (pow2 strides alias HBM banks; gather reads 1024B/row via
elem_step). The psqt term (8 tiny cols) is computed exactly on the host and
passed as a per-sample scalar. Per 128-sample tile and perspective: 4
dma_gather calls of 1024 rows (8 features x 128 samples, feature-major).
Reduce on PE: identity-matmul DoubleRow PSUM accumulate (exact), ft_bias
added in f32 after the merge.

Key scheduling constraint (v5): dma_gather descriptors are generated by
GpSimd(Q7), which a DVE op in 2-port perf mode (tensor_scalar / copy / cast)
FULLY BLOCKS via the shared SBUF port — starving the gather pipeline. All
large elementwise work therefore uses only tensor_tensor /
scalar_tensor_tensor (1-port, never contend) with constant tiles, and the
Activation engine for copies; measured gather-only rate is ~3.96us per
1024x1KB gather vs ~5.2us with contending ops in flight.
"""
import numpy as np
import ml_dtypes

import concourse.bacc as bacc
import concourse.bass as bass
import concourse.tile as tile
import concourse.mybir as mybir
from concourse.bass_utils import run_bass_kernel_spmd
from concourse.masks import make_identity
from concourse.library_config import mlp

F32 = mybir.dt.float32
BF16 = mybir.dt.bfloat16
FP8 = mybir.dt.float8e4
I16 = mybir.dt.int16

V = 22528          # ft table rows
FT = 1024          # ft embedding dim
PSQT = 8           # psqt buckets (host-side now)
E = 1024           # gathered row length in fp8 bytes (256B-aligned)
STEP = 1280        # table row stride in bytes (non-pow2 to spread HBM banks)
B = 8192
FEATS = 32         # features per bag
NCORES = 8
BC = B // NCORES   # samples per core
P = 128            # partitions
T = BC // P        # sample tiles per core (8)
NI = 1024          # rows per dma_gather (FPG features x 128 samples)
FPG = NI // P      # features per gather (8)
GPB = FEATS // FPG  # gathers per bag per tile (4)
L2 = 15
MIN = mybir.AluOpType.min
MAX = mybir.AluOpType.max
MULT = mybir.AluOpType.mult
ADD = mybir.AluOpType.add

_CACHE = {}


def _build_nc(reps=1, nq=2, gbufs=14):
    # SWDGE descriptor ring must hold one full gather (16B per descriptor)
    scratch = max(16384, NI * 16)
    nc = bacc.Bacc("TRN2", target_bir_lowering=False, debug=False,
                   num_swdge_queues=nq, dynamic_dma_scratch_size=scratch)

    tbl = nc.dram_tensor("tbl", [V, STEP], FP8, kind="ExternalInput")
    idx = nc.dram_tensor("idx", [P, T * 2 * GPB * (NI // 16)], I16,
                         kind="ExternalInput")
    m0 = nc.dram_tensor("m0", [P, T * 128], F32, kind="ExternalInput")
    m1 = nc.dram_tensor("m1", [P, T * 256], F32, kind="ExternalInput")
    m8 = nc.dram_tensor("m8", [P, T * 8], F32, kind="ExternalInput")
    psel_d = nc.dram_tensor("psel", [P, T], F32, kind="ExternalInput")
    cbias = nc.dram_tensor("cbias", [1, FT], F32, kind="ExternalInput")
    w0T = nc.dram_tensor("w0T", [P, FT], F32, kind="ExternalInput")
    w1T = nc.dram_tensor("w1T", [32, 256], F32, kind="ExternalInput")
    w2T = nc.dram_tensor("w2T", [32, 8], F32, kind="ExternalInput")
    b0 = nc.dram_tensor("b0", [1, 128], F32, kind="ExternalInput")
    b1 = nc.dram_tensor("b1", [1, 256], F32, kind="ExternalInput")
    b2 = nc.dram_tensor("b2", [1, 8], F32, kind="ExternalInput")
    out = nc.dram_tensor("out", [BC, 1], F32, kind="ExternalOutput")

    with tile.TileContext(nc) as tc:
        with tc.tile_pool(name="const", bufs=1) as cp, \
             tc.tile_pool(name="gat", bufs=gbufs) as gpool, \
             tc.tile_pool(name="accs", bufs=2) as apool, \
             tc.tile_pool(name="small", bufs=2) as spool, \
             tc.tile_pool(name="psum", bufs=2, space="PSUM") as pp, \
             tc.tile_pool(name="psred", bufs=2, space="PSUM") as pred, \
             tc.tile_pool(name="psacc", bufs=2, space="PSUM") as ppacc:

            nc.gpsimd.load_library(mlp)

            # ---- constants, loaded once ----
            idx_sb = cp.tile([P, T * 2 * GPB * (NI // 16)], I16)
            nc.sync.dma_start(idx_sb[:], idx[:])
            ident = cp.tile([P, P], F32)
            make_identity(nc, ident[:])
            id2 = cp.tile([P, 2 * P], FP8)
            nc.scalar.copy(id2[:, 0:P], ident[:])
            nc.scalar.copy(id2[:, P:2 * P], ident[:])
            cbias_sb = cp.tile([P, FT], F32)
            nc.sync.dma_start(cbias_sb[:], cbias[:].to_broadcast((P, FT)))
            w0T_sb = cp.tile([P, FT], F32)
            nc.sync.dma_start(w0T_sb[:], w0T[:])
            w1T_sb = cp.tile([32, 256], F32)
            nc.sync.dma_start(w1T_sb[:], w1T[:])
            w2T_sb = cp.tile([32, 8], F32)
            nc.sync.dma_start(w2T_sb[:], w2T[:])
            b0_sb = cp.tile([P, 128], F32)
            nc.sync.dma_start(b0_sb[:], b0[:].to_broadcast((P, 128)))
            b1_sb = cp.tile([P, 256], F32)
            nc.sync.dma_start(b1_sb[:], b1[:].to_broadcast((P, 256)))
            b2_sb = cp.tile([P, 8], F32)
            nc.sync.dma_start(b2_sb[:], b2[:].to_broadcast((P, 8)))
            # all-tile bucket one-hot masks + host-side psqt, loaded once
            m0_all = cp.tile([P, T * 128], F32)
            nc.sync.dma_start(m0_all[:], m0[:])
            m1_all = cp.tile([P, T * 256], F32)
            nc.sync.dma_start(m1_all[:], m1[:])
            m8_all = cp.tile([P, T * 8], F32)
            nc.sync.dma_start(m8_all[:], m8[:])
            psel_all = cp.tile([P, T], F32)
            nc.sync.dma_start(psel_all[:], psel_d[:])
            # constant tiles for tensor_tensor-class clips (these DVE op
            # classes never block GpSimd descriptor generation)
            c127 = cp.tile([P, 512], F32)
            nc.vector.memset(c127[:], 127.0)
            cinv64 = cp.tile([P, 32], F32)
            nc.vector.memset(cinv64[:], 1.0 / 64.0)
            cz = cp.tile([P, 2], F32)
            nc.vector.memset(cz[:], 0.0)

            for t in [t for _ in range(reps) for t in range(T)]:
                rows = slice(t * P, (t + 1) * P)
                m0_t = m0_all[:, t * 128:(t + 1) * 128]
                m1_t = m1_all[:, t * 256:(t + 1) * 256]
                m8_t = m8_all[:, t * 8:(t + 1) * 8]

                # ---- gather + reduce both bags ----
                accs = []
                for bag in range(2):
                    acc = apool.tile([P, FT], F32,
                                     tag="acc_w" if bag == 0 else "acc_b")
                    bufs = []
                    for g in range(GPB):
                        j = (t * 2 + bag) * GPB + g
                        col = j * (NI // 16)
                        buf = gpool.tile([P, FPG, E], FP8, tag="gather")
                        nc.gpsimd.dma_gather(
                            buf[:], tbl[:, 0:E], idx_sb[:, col:col + NI // 16],
                            NI, NI, E, elem_step=STEP,
                            transpose=False, queue_num=j % nq)
                        bufs.append(buf)

                    # PE: all gathers reduced via identity matmuls (fp8,
                    # DoubleRow), accumulated exactly in f32 PSUM
                    psA = pred.tile([P, 512], F32, tag="psA", space="PSUM")
                    psB = pred.tile([P, 512], F32, tag="psB", space="PSUM")
                    n_mm = GPB * FPG // 2
                    i_mm = 0
                    i2v = id2[:].rearrange("p (two f) -> p two f", two=2)
                    DR = mybir.MatmulPerfMode.DoubleRow
                    for g in range(GPB):
                        for flp in range(0, FPG, 2):
                            st = i_mm == 0
                            sp = i_mm == n_mm - 1
                            nc.tensor.matmul(
                                psA[:], lhsT=i2v,
                                rhs=bufs[g][:, flp:flp + 2, 0:512],
                                start=st, stop=sp, perf_mode=DR)
                            nc.tensor.matmul(
                                psB[:], lhsT=i2v,
                                rhs=bufs[g][:, flp:flp + 2, 512:1024],
                                start=st, stop=sp, perf_mode=DR)
                            i_mm += 1

                    # merge + exact f32 bias (tensor_tensor class)
                    nc.vector.tensor_add(acc[:, 0:512], psA[:],
                                         cbias_sb[:, 0:512])
                    nc.vector.tensor_add(acc[:, 512:1024], psB[:],
                                         cbias_sb[:, 512:1024])
                    accs.append(acc)
                acc_stm, acc_opp = accs

                # ---- pairwise: clip both halves, multiply, in place ----
                H = FT // 2
                ft_halves = []
                for acc in (acc_stm, acc_opp):
                    # clip(x,0,127) = (x max 0.0) min c127 — stt class
                    nc.vector.scalar_tensor_tensor(
                        acc[:, 0:H], acc[:, 0:H], 0.0, c127[:],
                        op0=MAX, op1=MIN)
                    nc.vector.scalar_tensor_tensor(
                        acc[:, H:FT], acc[:, H:FT], 0.0, c127[:],
                        op0=MAX, op1=MIN)
                    nc.vector.tensor_mul(acc[:, 0:H], acc[:, 0:H], acc[:, H:FT])
                    ft_halves.append(acc[:, 0:H])

                # ---- fc0: transpose ft tiles, matmul all stacks ----
                mm = ppacc.tile([P, 256], F32, tag="mm", space="PSUM")
                o0p = mm[:, 0:128]
                for k in range(8):
                    col = (k % 4) * P
                    tp = pp.tile([P, P], F32, tag="tpose", space="PSUM")
                    nc.tensor.transpose(tp[:], ft_halves[k // 4][:, col:col + P], ident[:])
                    ftT = spool.tile([P, P], F32, tag="ftT")
                    nc.scalar.copy(ftT[:], tp[:])
                    nc.tensor.matmul(
                        o0p, lhsT=ftT[:], rhs=w0T_sb[:, k * P:(k + 1) * P],
                        start=(k == 0), stop=(k == 7))

                # ---- fc0 bias + mask select ----
                o0m = spool.tile([P, 128], F32, tag="o0m")
                nc.vector.tensor_add(o0m[:], o0p, b0_sb[:])
                nc.vector.tensor_mul(o0m[:], o0m[:], m0_t)
                o0h = spool.tile([P, 64], F32, tag="o0h")
                nc.vector.tensor_add(o0h[:], o0m[:, 0:64], o0m[:, 64:128])
                nc.vector.tensor_add(o0h[:, 0:32], o0h[:, 0:32], o0h[:, 32:64])
                o0 = spool.tile([P, 16], F32, tag="o0")
                nc.vector.tensor_add(o0[:], o0h[:, 0:16], o0h[:, 16:32])

                # ---- slab activations (tt/stt classes + ACT copy only) ----
                slab = spool.tile([P, 32], F32, tag="slab")
                nc.scalar.copy(slab[:, 30:32], cz[:])
                sq = spool.tile([P, L2], F32, tag="sq")
                nc.vector.tensor_mul(sq[:], o0[:, 0:L2], o0[:, 0:L2])
                # sqr slab: (sq * 1/2^19) min 127; sq >= 0 so no lower clip
                nc.vector.scalar_tensor_tensor(
                    slab[:, 0:L2], sq[:], 1.0 / 524288.0, c127[:, 0:L2],
                    op0=MULT, op1=MIN)
                # rel slab: ((o0 max 0) * 1/64) min 127
                nc.vector.scalar_tensor_tensor(
                    slab[:, L2:2 * L2], o0[:, 0:L2], 0.0, cinv64[:, 0:L2],
                    op0=MAX, op1=MULT)
                nc.vector.tensor_tensor(
                    slab[:, L2:2 * L2], slab[:, L2:2 * L2], c127[:, 0:L2], op=MIN)

                # ---- fc1 ----
                tpf = pp.tile([P, P], F32, tag="tpose", space="PSUM")
                tps = tpf[0:32, :]
                nc.tensor.transpose(tps, slab[:], ident[:])
                slabT = spool.tile([32, P], F32, tag="slabT")
                nc.scalar.copy(slabT[:], tps)
                o1p = mm[:, 0:256]
                nc.tensor.matmul(o1p, lhsT=slabT[:], rhs=w1T_sb[:], start=True, stop=True)
                o1m = spool.tile([P, 256], F32, tag="o1m")
                nc.vector.tensor_add(o1m[:], o1p, b1_sb[:])
                nc.vector.tensor_mul(o1m[:], o1m[:], m1_t)
                o1h = spool.tile([P, 128], F32, tag="o1h")
                nc.vector.tensor_add(o1h[:], o1m[:, 0:128], o1m[:, 128:256])
                nc.vector.tensor_add(o1h[:, 0:64], o1h[:, 0:64], o1h[:, 64:128])
                o1 = spool.tile([P, 32], F32, tag="o1")
                nc.vector.tensor_add(o1[:], o1h[:, 0:32], o1h[:, 32:64])
                # ac1 = ((o1 max 0) * 1/64) min 127
                nc.vector.scalar_tensor_tensor(
                    o1[:], o1[:], 0.0, cinv64[:], op0=MAX, op1=MULT)
                nc.vector.tensor_tensor(o1[:], o1[:], c127[:, 0:32], op=MIN)

                # ---- fc2 ----
                tpg = pp.tile([P, P], F32, tag="tpose", space="PSUM")
                tpa = tpg[0:32, :]
                nc.tensor.transpose(tpa, o1[:], ident[:])
                ac1T = spool.tile([32, P], F32, tag="ac1T")
                nc.scalar.copy(ac1T[:], tpa)
                o2p = mm[:, 0:8]
                nc.tensor.matmul(o2p, lhsT=ac1T[:], rhs=w2T_sb[:], start=True, stop=True)
                o2m = spool.tile([P, 8], F32, tag="o2m")
                nc.vector.tensor_add(o2m[:], o2p, b2_sb[:])
                nc.vector.tensor_mul(o2m[:], o2m[:], m8_t)
                o2h = spool.tile([P, 4], F32, tag="o2h")
                nc.vector.tensor_add(o2h[:], o2m[:, 0:4], o2m[:, 4:8])
                nc.vector.tensor_add(o2h[:, 0:2], o2h[:, 0:2], o2h[:, 2:4])
                res = spool.tile([P, 1], F32, tag="res")
                nc.vector.tensor_add(res[:], o2h[:, 0:1], o2h[:, 1:2])

                # ---- skip + host-side psqt + output ----
                # res += o0[:,15] * (9600/8128/16)  (stt class)
                nc.vector.scalar_tensor_tensor(
                    res[:], o0[:, L2:16], 9600.0 / 8128.0 / 16.0, res[:],
                    op0=MULT, op1=ADD)
                nc.vector.tensor_add(res[:], res[:], psel_all[:, t:t + 1])
                nc.sync.dma_start(out[rows, :], res[:])

    nc.compile()
    return nc


def _prep_inputs(inputs):
    """Host-side prep: fp8 table, wrapped int16 gather indices (stm-swapped),
    transposed/prescaled weights, bucket one-hot masks, and the exact psqt
    term (tiny [V,8] table; computed here so the device gathers 1024B rows)."""
    ft_w = np.asarray(inputs["ft_w"], dtype=np.float32)
    psqt_w = np.asarray(inputs["psqt_w"], dtype=np.float32)
    ft_bias = np.asarray(inputs["ft_bias"], dtype=np.float32)
    tbl = np.zeros((V, STEP), dtype=ml_dtypes.float8_e4m3fn)
    tbl[:, :FT] = ft_w.astype(ml_dtypes.float8_e4m3fn)
    cbias = ft_bias.reshape(1, FT)

    fc0_w = np.asarray(inputs["fc0_w"], dtype=np.float32)  # [8,16,1024]
    fc1_w = np.asarray(inputs["fc1_w"], dtype=np.float32)  # [8,32,32]
    fc2_w = np.asarray(inputs["fc2_w"], dtype=np.float32)  # [8,1,32]
    a = fc0_w.transpose(2, 0, 1).reshape(FT, 128) * (1.0 / 128.0)   # [h, (s,j)]
    w0T = np.ascontiguousarray(
        a.reshape(8, 128, 128).transpose(1, 0, 2).reshape(128, FT))
    w1T = np.ascontiguousarray(fc1_w.transpose(2, 0, 1).reshape(32, 256))
    w2T = np.ascontiguousarray(fc2_w[:, 0, :].T * (1.0 / 16.0))     # [32, 8]
    b0 = np.asarray(inputs["fc0_b"], np.float32).reshape(1, 128)
    b1 = np.asarray(inputs["fc1_b"], np.float32).reshape(1, 256)
    b2 = np.asarray(inputs["fc2_b"], np.float32).reshape(1, 8) * (1.0 / 16.0)

    w_feats = np.asarray(inputs["w_feats"]).astype(np.int64)
    b_feats = np.asarray(inputs["b_feats"]).astype(np.int64)
    stm = np.asarray(inputs["stm"]).astype(np.float32)
    bucket = np.asarray(inputs["bucket"]).astype(np.int64)

    # exact host-side psqt: (psqt_stm - psqt_opp)[b, bucket] / 32
    # (the /2 from the reference plus the final /16 folded together)
    psqt_wb = psqt_w[w_feats.reshape(B, FEATS)].sum(axis=1)  # [B, 8]
    psqt_bb = psqt_w[b_feats.reshape(B, FEATS)].sum(axis=1)
    stm_col = stm.reshape(B, 1)
    psqt_stm = psqt_wb * (1 - stm_col) + psqt_bb * stm_col
    psqt_opp = psqt_bb * (1 - stm_col) + psqt_wb * stm_col
    bidx = np.arange(B)
    psel = (psqt_stm[bidx, bucket] - psqt_opp[bidx, bucket]) / 32.0  # [B]

    w_feats16 = w_feats.astype(np.int16)
    b_feats16 = b_feats.astype(np.int16)

    in_maps = []
    for c in range(NCORES):
        s = slice(c * BC, (c + 1) * BC)
        wf = w_feats16[c * BC * FEATS:(c + 1) * BC * FEATS].reshape(T, P, FEATS)
        bf = b_feats16[c * BC * FEATS:(c + 1) * BC * FEATS].reshape(T, P, FEATS)
        st_tp = stm[s].reshape(T, P, 1) > 0.5
        f_stm = np.where(st_tp, bf, wf)
        f_opp = np.where(st_tp, wf, bf)
        # gather j=(t*2+bag)*GPB+g covers features FPG*g..FPG*(g+1)-1,
        # feature-major: local id i = f_local*128 + sample
        blocks = np.empty((T, 2, GPB, NI), np.int16)
        for bag, arr in enumerate((f_stm, f_opp)):
            ak = arr.reshape(T, P, GPB, FPG).transpose(0, 2, 3, 1)  # [T,GPB,FPG,P]
            blocks[:, bag, :, :] = ak.reshape(T, GPB, NI)
        flat = blocks.reshape(T * 2 * GPB, NI)
        wrapped = flat.reshape(-1, NI // 16, 16).transpose(0, 2, 1)
        idx_arr = np.zeros((P, T * 2 * GPB * (NI // 16)), np.int16)
        cols = wrapped.transpose(1, 0, 2).reshape(16, -1)
        for g in range(8):
            idx_arr[g * 16:(g + 1) * 16, :] = cols

        bk = bucket[s]
        m0 = (bk[:, None] == (np.arange(128) // 16)).astype(np.float32)
        m1 = (bk[:, None] == (np.arange(256) // 32)).astype(np.float32)
        m8 = (bk[:, None] == np.arange(8)).astype(np.float32)
        # SBUF layout: partition = sample-in-tile, cols = tile-major blocks
        m0 = np.ascontiguousarray(
            m0.reshape(T, P, 128).transpose(1, 0, 2).reshape(P, T * 128))
        m1 = np.ascontiguousarray(
            m1.reshape(T, P, 256).transpose(1, 0, 2).reshape(P, T * 256))
        m8 = np.ascontiguousarray(
            m8.reshape(T, P, 8).transpose(1, 0, 2).reshape(P, T * 8))
        psel_c = np.ascontiguousarray(
            psel[s].astype(np.float32).reshape(T, P).T)  # [P, T]
        in_maps.append({
            "tbl": tbl, "idx": idx_arr, "cbias": cbias,
            "m0": m0, "m1": m1, "m8": m8, "psel": psel_c,
            "w0T": w0T, "w1T": w1T, "w2T": w2T,
            "b0": b0, "b1": b1, "b2": b2,
        })
    return in_maps


def kernel(**inputs) -> np.ndarray:
    if "nc" not in _CACHE:
        _CACHE["nc"] = _build_nc()
    nc = _CACHE["nc"]
    in_maps = _prep_inputs(inputs)
    r = run_bass_kernel_spmd(nc, in_maps, core_ids=list(range(NCORES)))
    return np.concatenate([r.results[c]["out"][:, 0] for c in range(NCORES)])


# revision 8
# speedup vs baseline: 317.8349x; 1.0459x over previous
"""NNUE (HalfKA) forward kernel for Trainium2, data-parallel over batch on 8 cores.

v5: fp8(e4m3) embedding table, rows hold the 1024 ft columns at a 1280B
non-pow2 row stride (pow2 strides alias HBM banks; gather reads 1024B/row via
elem_step). The psqt term (8 tiny cols) is computed exactly on the host and
passed as a per-sample scalar. Per 128-sample tile and perspective: 4
dma_gather calls of 1024 rows (8 features x 128 samples, feature-major).
Reduce on PE: identity-matmul DoubleRow PSUM accumulate (exact), ft_bias
added in f32 after the merge.

Key scheduling constraint (v5): dma_gather descriptors are generated by
GpSimd(Q7), which a DVE op in 2-port perf mode (tensor_scalar / copy / cast)
FULLY BLOCKS via the shared SBUF port — starving the gather pipeline. All
large elementwise work therefore uses only tensor_tensor /
scalar_tensor_tensor (1-port, never contend) with constant tiles, and the
Activation engine for copies; measured gather-only rate is ~3.96us per
1024x1KB gather vs ~5.2us with contending ops in flight.
"""
import numpy as np
import ml_dtypes

import concourse.bacc as bacc
import concourse.bass as bass
import concourse.tile as tile
import concourse.mybir as mybir
from concourse.bass_utils import run_bass_kernel_spmd
from concourse.masks import make_identity
from concourse.library_config import mlp

F32 = mybir.dt.float32
BF16 = mybir.dt.bfloat16
FP8 = mybir.dt.float8e4
I16 = mybir.dt.int16

V = 22528          # ft table rows
FT = 1024          # ft embedding dim
PSQT = 8           # psqt buckets (host-side now)
E = 1024           # gathered row length in fp8 bytes (256B-aligned)
STEP = 1280        # table row stride in bytes (non-pow2 to spread HBM banks)
B = 8192
FEATS = 32         # features per bag
NCORES = 8
BC = B // NCORES   # samples per core
P = 128            # partitions
T = BC // P        # sample tiles per core (8)
NI = 1024          # rows per dma_gather (FPG features x 128 samples)
FPG = NI // P      # features per gather (8)
GPB = FEATS // FPG  # gathers per bag per tile (4)
L2 = 15
MIN = mybir.AluOpType.min
MAX = mybir.AluOpType.max
MULT = mybir.AluOpType.mult
ADD = mybir.AluOpType.add

_CACHE = {}


def _build_nc(reps=1, nq=3, gbufs=14):
    # SWDGE descriptor ring must hold one full gather (16B per descriptor)
    scratch = max(16384, NI * 16)
    nc = bacc.Bacc("TRN2", target_bir_lowering=False, debug=False,
                   num_swdge_queues=nq, dynamic_dma_scratch_size=scratch)

    tbl = nc.dram_tensor("tbl", [V, STEP], FP8, kind="ExternalInput")
    idx = nc.dram_tensor("idx", [P, T * 2 * GPB * (NI // 16)], I16,
                         kind="ExternalInput")
    m0 = nc.dram_tensor("m0", [P, T * 128], F32, kind="ExternalInput")
    m1 = nc.dram_tensor("m1", [P, T * 256], F32, kind="ExternalInput")
    m8 = nc.dram_tensor("m8", [P, T * 8], F32, kind="ExternalInput")
    psel_d = nc.dram_tensor("psel", [P, T], F32, kind="ExternalInput")
    cbias = nc.dram_tensor("cbias", [1, FT], F32, kind="ExternalInput")
    w0T = nc.dram_tensor("w0T", [P, FT], F32, kind="ExternalInput")
    w1T = nc.dram_tensor("w1T", [32, 256], F32, kind="ExternalInput")
    w2T = nc.dram_tensor("w2T", [32, 8], F32, kind="ExternalInput")
    b0 = nc.dram_tensor("b0", [1, 128], F32, kind="ExternalInput")
    b1 = nc.dram_tensor("b1", [1, 256], F32, kind="ExternalInput")
    b2 = nc.dram_tensor("b2", [1, 8], F32, kind="ExternalInput")
    out = nc.dram_tensor("out", [BC, 1], F32, kind="ExternalOutput")

    with tile.TileContext(nc) as tc:
        with tc.tile_pool(name="const", bufs=1) as cp, \
             tc.tile_pool(name="gat", bufs=gbufs) as gpool, \
             tc.tile_pool(name="accs", bufs=2) as apool, \
             tc.tile_pool(name="small", bufs=2) as spool, \
             tc.tile_pool(name="psum", bufs=2, space="PSUM") as pp, \
             tc.tile_pool(name="psred", bufs=2, space="PSUM") as pred, \
             tc.tile_pool(name="psacc", bufs=2, space="PSUM") as ppacc:

            nc.gpsimd.load_library(mlp)

            # ---- constants, loaded once ----
            idx_sb = cp.tile([P, T * 2 * GPB * (NI // 16)], I16)
            nc.sync.dma_start(idx_sb[:], idx[:])
            ident = cp.tile([P, P], F32)
            make_identity(nc, ident[:])
            id2 = cp.tile([P, 2 * P], FP8)
            nc.scalar.copy(id2[:, 0:P], ident[:])
            nc.scalar.copy(id2[:, P:2 * P], ident[:])
            cbias_sb = cp.tile([P, FT], F32)
            nc.sync.dma_start(cbias_sb[:], cbias[:].to_broadcast((P, FT)))
            w0T_sb = cp.tile([P, FT], F32)
            nc.sync.dma_start(w0T_sb[:], w0T[:])
            w1T_sb = cp.tile([32, 256], F32)
            nc.sync.dma_start(w1T_sb[:], w1T[:])
            w2T_sb = cp.tile([32, 8], F32)
            nc.sync.dma_start(w2T_sb[:], w2T[:])
            b0_sb = cp.tile([P, 128], F32)
            nc.sync.dma_start(b0_sb[:], b0[:].to_broadcast((P, 128)))
            b1_sb = cp.tile([P, 256], F32)
            nc.sync.dma_start(b1_sb[:], b1[:].to_broadcast((P, 256)))
            b2_sb = cp.tile([P, 8], F32)
            nc.sync.dma_start(b2_sb[:], b2[:].to_broadcast((P, 8)))
            # all-tile bucket one-hot masks + host-side psqt, loaded once
            m0_all = cp.tile([P, T * 128], F32)
            nc.sync.dma_start(m0_all[:], m0[:])
            m1_all = cp.tile([P, T * 256], F32)
            nc.sync.dma_start(m1_all[:], m1[:])
            m8_all = cp.tile([P, T * 8], F32)
            nc.sync.dma_start(m8_all[:], m8[:])
            psel_all = cp.tile([P, T], F32)
            nc.sync.dma_start(psel_all[:], psel_d[:])
            # constant tiles for tensor_tensor-class clips (these DVE op
            # classes never block GpSimd descriptor generation)
            c127 = cp.tile([P, 512], F32)
            nc.vector.memset(c127[:], 127.0)
            cinv64 = cp.tile([P, 32], F32)
            nc.vector.memset(cinv64[:], 1.0 / 64.0)
            cz = cp.tile([P, 2], F32)
            nc.vector.memset(cz[:], 0.0)

            for t in [t for _ in range(reps) for t in range(T)]:
                rows = slice(t * P, (t + 1) * P)
                m0_t = m0_all[:, t * 128:(t + 1) * 128]
                m1_t = m1_all[:, t * 256:(t + 1) * 256]
                m8_t = m8_all[:, t * 8:(t + 1) * 8]

                # ---- gather + reduce both bags ----
                accs = []
                for bag in range(2):
                    acc = apool.tile([P, FT], F32,
                                     tag="acc_w" if bag == 0 else "acc_b")
                    bufs = []
                    for g in range(GPB):
                        j = (t * 2 + bag) * GPB + g
                        col = j * (NI // 16)
                        buf = gpool.tile([P, FPG, E], FP8, tag="gather")
                        nc.gpsimd.dma_gather(
                            buf[:], tbl[:, 0:E], idx_sb[:, col:col + NI // 16],
                            NI, NI, E, elem_step=STEP,
                            transpose=False, queue_num=j % nq)
                        bufs.append(buf)

                    # PE: all gathers reduced via identity matmuls (fp8,
                    # DoubleRow), accumulated exactly in f32 PSUM
                    psA = pred.tile([P, 512], F32, tag="psA", space="PSUM")
                    psB = pred.tile([P, 512], F32, tag="psB", space="PSUM")
                    n_mm = GPB * FPG // 2
                    i_mm = 0
                    i2v = id2[:].rearrange("p (two f) -> p two f", two=2)
                    DR = mybir.MatmulPerfMode.DoubleRow
                    for g in range(GPB):
                        for flp in range(0, FPG, 2):
                            st = i_mm == 0
                            sp = i_mm == n_mm - 1
                            nc.tensor.matmul(
                                psA[:], lhsT=i2v,
                                rhs=bufs[g][:, flp:flp + 2, 0:512],
                                start=st, stop=sp, perf_mode=DR)
                            nc.tensor.matmul(
                                psB[:], lhsT=i2v,
                                rhs=bufs[g][:, flp:flp + 2, 512:1024],
                                start=st, stop=sp, perf_mode=DR)
                            i_mm += 1

                    # merge + exact f32 bias (tensor_tensor class)
                    nc.vector.tensor_add(acc[:, 0:512], psA[:],
                                         cbias_sb[:, 0:512])
                    nc.vector.tensor_add(acc[:, 512:1024], psB[:],
                                         cbias_sb[:, 512:1024])
                    accs.append(acc)
                acc_stm, acc_opp = accs

                # ---- pairwise: clip both halves, multiply, in place ----
                H = FT // 2
                ft_halves = []
                for acc in (acc_stm, acc_opp):
                    # clip(x,0,127) = (x max 0.0) min c127 — stt class
                    nc.vector.scalar_tensor_tensor(
                        acc[:, 0:H], acc[:, 0:H], 0.0, c127[:],
                        op0=MAX, op1=MIN)
                    nc.vector.scalar_tensor_tensor(
                        acc[:, H:FT], acc[:, H:FT], 0.0, c127[:],
                        op0=MAX, op1=MIN)
                    nc.vector.tensor_mul(acc[:, 0:H], acc[:, 0:H], acc[:, H:FT])
                    ft_halves.append(acc[:, 0:H])

                # ---- fc0: transpose ft tiles, matmul all stacks ----
                mm = ppacc.tile([P, 256], F32, tag="mm", space="PSUM")
                o0p = mm[:, 0:128]
                for k in range(8):
                    col = (k % 4) * P
                    tp = pp.tile([P, P], F32, tag="tpose", space="PSUM")
                    nc.tensor.transpose(tp[:], ft_halves[k // 4][:, col:col + P], ident[:])
                    ftT = spool.tile([P, P], F32, tag="ftT")
                    nc.scalar.copy(ftT[:], tp[:])
                    nc.tensor.matmul(
                        o0p, lhsT=ftT[:], rhs=w0T_sb[:, k * P:(k + 1) * P],
                        start=(k == 0), stop=(k == 7))

                # ---- fc0 bias + mask select ----
                o0m = spool.tile([P, 128], F32, tag="o0m")
                nc.vector.tensor_add(o0m[:], o0p, b0_sb[:])
                nc.vector.tensor_mul(o0m[:], o0m[:], m0_t)
                o0h = spool.tile([P, 64], F32, tag="o0h")
                nc.vector.tensor_add(o0h[:], o0m[:, 0:64], o0m[:, 64:128])
                nc.vector.tensor_add(o0h[:, 0:32], o0h[:, 0:32], o0h[:, 32:64])
                o0 = spool.tile([P, 16], F32, tag="o0")
                nc.vector.tensor_add(o0[:], o0h[:, 0:16], o0h[:, 16:32])

                # ---- slab activations (tt/stt classes + ACT copy only) ----
                slab = spool.tile([P, 32], F32, tag="slab")
                nc.scalar.copy(slab[:, 30:32], cz[:])
                sq = spool.tile([P, L2], F32, tag="sq")
                nc.vector.tensor_mul(sq[:], o0[:, 0:L2], o0[:, 0:L2])
                # sqr slab: (sq * 1/2^19) min 127; sq >= 0 so no lower clip
                nc.vector.scalar_tensor_tensor(
                    slab[:, 0:L2], sq[:], 1.0 / 524288.0, c127[:, 0:L2],
                    op0=MULT, op1=MIN)
                # rel slab: ((o0 max 0) * 1/64) min 127
                nc.vector.scalar_tensor_tensor(
                    slab[:, L2:2 * L2], o0[:, 0:L2], 0.0, cinv64[:, 0:L2],
                    op0=MAX, op1=MULT)
                nc.vector.tensor_tensor(
                    slab[:, L2:2 * L2], slab[:, L2:2 * L2], c127[:, 0:L2], op=MIN)

                # ---- fc1 ----
                tpf = pp.tile([P, P], F32, tag="tpose", space="PSUM")
                tps = tpf[0:32, :]
                nc.tensor.transpose(tps, slab[:], ident[:])
                slabT = spool.tile([32, P], F32, tag="slabT")
                nc.scalar.copy(slabT[:], tps)
                o1p = mm[:, 0:256]
                nc.tensor.matmul(o1p, lhsT=slabT[:], rhs=w1T_sb[:], start=True, stop=True)
                o1m = spool.tile([P, 256], F32, tag="o1m")
                nc.vector.tensor_add(o1m[:], o1p, b1_sb[:])
                nc.vector.tensor_mul(o1m[:], o1m[:], m1_t)
                o1h = spool.tile([P, 128], F32, tag="o1h")
                nc.vector.tensor_add(o1h[:], o1m[:, 0:128], o1m[:, 128:256])
                nc.vector.tensor_add(o1h[:, 0:64], o1h[:, 0:64], o1h[:, 64:128])
                o1 = spool.tile([P, 32], F32, tag="o1")
                nc.vector.tensor_add(o1[:], o1h[:, 0:32], o1h[:, 32:64])
                # ac1 = ((o1 max 0) * 1/64) min 127
                nc.vector.scalar_tensor_tensor(
                    o1[:], o1[:], 0.0, cinv64[:], op0=MAX, op1=MULT)
                nc.vector.tensor_tensor(o1[:], o1[:], c127[:, 0:32], op=MIN)

                # ---- fc2 ----
                tpg = pp.tile([P, P], F32, tag="tpose", space="PSUM")
                tpa = tpg[0:32, :]
                nc.tensor.transpose(tpa, o1[:], ident[:])
                ac1T = spool.tile([32, P], F32, tag="ac1T")
                nc.scalar.copy(ac1T[:], tpa)
                o2p = mm[:, 0:8]
                nc.tensor.matmul(o2p, lhsT=ac1T[:], rhs=w2T_sb[:], start=True, stop=True)
                o2m = spool.tile([P, 8], F32, tag="o2m")
                nc.vector.tensor_add(o2m[:], o2p, b2_sb[:])
                nc.vector.tensor_mul(o2m[:], o2m[:], m8_t)
                o2h = spool.tile([P, 4], F32, tag="o2h")
                nc.vector.tensor_add(o2h[:], o2m[:, 0:4], o2m[:, 4:8])
                nc.vector.tensor_add(o2h[:, 0:2], o2h[:, 0:2], o2h[:, 2:4])
                res = spool.tile([P, 1], F32, tag="res")
                nc.vector.tensor_add(res[:], o2h[:, 0:1], o2h[:, 1:2])

                # ---- skip + host-side psqt + output ----
                # res += o0[:,15] * (9600/8128/16)  (stt class)
                nc.vector.scalar_tensor_tensor(
                    res[:], o0[:, L2:16], 9600.0 / 8128.0 / 16.0, res[:],
                    op0=MULT, op1=ADD)
                nc.vector.tensor_add(res[:], res[:], psel_all[:, t:t + 1])
                nc.sync.dma_start(out[rows, :], res[:])

    nc.compile()
    return nc


def _prep_inputs(inputs):
    """Host-side prep: fp8 table, wrapped int16 gather indices (stm-swapped),
    transposed/prescaled weights, bucket one-hot masks, and the exact psqt
    term (tiny [V,8] table; computed here so the device gathers 1024B rows)."""
    ft_w = np.asarray(inputs["ft_w"], dtype=np.float32)
    psqt_w = np.asarray(inputs["psqt_w"], dtype=np.float32)
    ft_bias = np.asarray(inputs["ft_bias"], dtype=np.float32)
    tbl = np.zeros((V, STEP), dtype=ml_dtypes.float8_e4m3fn)
    tbl[:, :FT] = ft_w.astype(ml_dtypes.float8_e4m3fn)
    cbias = ft_bias.reshape(1, FT)

    fc0_w = np.asarray(inputs["fc0_w"], dtype=np.float32)  # [8,16,1024]
    fc1_w = np.asarray(inputs["fc1_w"], dtype=np.float32)  # [8,32,32]
    fc2_w = np.asarray(inputs["fc2_w"], dtype=np.float32)  # [8,1,32]
    a = fc0_w.transpose(2, 0, 1).reshape(FT, 128) * (1.0 / 128.0)   # [h, (s,j)]
    w0T = np.ascontiguousarray(
        a.reshape(8, 128, 128).transpose(1, 0, 2).reshape(128, FT))
    w1T = np.ascontiguousarray(fc1_w.transpose(2, 0, 1).reshape(32, 256))
    w2T = np.ascontiguousarray(fc2_w[:, 0, :].T * (1.0 / 16.0))     # [32, 8]
    b0 = np.asarray(inputs["fc0_b"], np.float32).reshape(1, 128)
    b1 = np.asarray(inputs["fc1_b"], np.float32).reshape(1, 256)
    b2 = np.asarray(inputs["fc2_b"], np.float32).reshape(1, 8) * (1.0 / 16.0)

    w_feats = np.asarray(inputs["w_feats"]).astype(np.int64)
    b_feats = np.asarray(inputs["b_feats"]).astype(np.int64)
    stm = np.asarray(inputs["stm"]).astype(np.float32)
    bucket = np.asarray(inputs["bucket"]).astype(np.int64)

    # exact host-side psqt: (psqt_stm - psqt_opp)[b, bucket] / 32
    # (the /2 from the reference plus the final /16 folded together)
    psqt_wb = psqt_w[w_feats.reshape(B, FEATS)].sum(axis=1)  # [B, 8]
    psqt_bb = psqt_w[b_feats.reshape(B, FEATS)].sum(axis=1)
    stm_col = stm.reshape(B, 1)
    psqt_stm = psqt_wb * (1 - stm_col) + psqt_bb * stm_col
    psqt_opp = psqt_bb * (1 - stm_col) + psqt_wb * stm_col
    bidx = np.arange(B)
    psel = (psqt_stm[bidx, bucket] - psqt_opp[bidx, bucket]) / 32.0  # [B]

    w_feats16 = w_feats.astype(np.int16)
    b_feats16 = b_feats.astype(np.int16)

    in_maps = []
    for c in range(NCORES):
        s = slice(c * BC, (c + 1) * BC)
        wf = w_feats16[c * BC * FEATS:(c + 1) * BC * FEATS].reshape(T, P, FEATS)
        bf = b_feats16[c * BC * FEATS:(c + 1) * BC * FEATS].reshape(T, P, FEATS)
        st_tp = stm[s].reshape(T, P, 1) > 0.5
        f_stm = np.where(st_tp, bf, wf)
        f_opp = np.where(st_tp, wf, bf)
        # gather j=(t*2+bag)*GPB+g covers features FPG*g..FPG*(g+1)-1,
        # feature-major: local id i = f_local*128 + sample
        blocks = np.empty((T, 2, GPB, NI), np.int16)
        for bag, arr in enumerate((f_stm, f_opp)):
            ak = arr.reshape(T, P, GPB, FPG).transpose(0, 2, 3, 1)  # [T,GPB,FPG,P]
            blocks[:, bag, :, :] = ak.reshape(T, GPB, NI)
        flat = blocks.reshape(T * 2 * GPB, NI)
        wrapped = flat.reshape(-1, NI // 16, 16).transpose(0, 2, 1)
        idx_arr = np.zeros((P, T * 2 * GPB * (NI // 16)), np.int16)
        cols = wrapped.transpose(1, 0, 2).reshape(16, -1)
        for g in range(8):
            idx_arr[g * 16:(g + 1) * 16, :] = cols

        bk = bucket[s]
        m0 = (bk[:, None] == (np.arange(128) // 16)).astype(np.float32)
        m1 = (bk[:, None] == (np.arange(256) // 32)).astype(np.float32)
        m8 = (bk[:, None] == np.arange(8)).astype(np.float32)
        # SBUF layout: partition = sample-in-tile, cols = tile-major blocks
        m0 = np.ascontiguousarray(
            m0.reshape(T, P, 128).transpose(1, 0, 2).reshape(P, T * 128))
        m1 = np.ascontiguousarray(
            m1.reshape(T, P, 256).transpose(1, 0, 2).reshape(P, T * 256))
        m8 = np.ascontiguousarray(
            m8.reshape(T, P, 8).transpose(1, 0, 2).reshape(P, T * 8))
        psel_c = np.ascontiguousarray(
            psel[s].astype(np.float32).reshape(T, P).T)  # [P, T]
        in_maps.append({
            "tbl": tbl, "idx": idx_arr, "cbias": cbias,
            "m0": m0, "m1": m1, "m8": m8, "psel": psel_c,
            "w0T": w0T, "w1T": w1T, "w2T": w2T,
            "b0": b0, "b1": b1, "b2": b2,
        })
    return in_maps


def kernel(**inputs) -> np.ndarray:
    if "nc" not in _CACHE:
        _CACHE["nc"] = _build_nc()
    nc = _CACHE["nc"]
    in_maps = _prep_inputs(inputs)
    r = run_bass_kernel_spmd(nc, in_maps, core_ids=list(range(NCORES)))
    return np.concatenate([r.results[c]["out"][:, 0] for c in range(NCORES)])


# revision 10
# speedup vs baseline: 318.8036x; 1.0030x over previous
"""NNUE (HalfKA) forward kernel for Trainium2, data-parallel over batch on 8 cores.

v5: fp8(e4m3) embedding table, rows hold the 1024 ft columns at a 1280B
non-pow2 row stride (pow2 strides alias HBM banks; gather reads 1024B/row via
elem_step). The psqt term (8 tiny cols) is computed exactly on the host and
passed as a per-sample scalar. Per 128-sample tile and perspective: 4
dma_gather calls of 1024 rows (8 features x 128 samples, feature-major).
Reduce on PE: identity-matmul DoubleRow PSUM accumulate (exact), ft_bias
added in f32 after the merge.

Key scheduling constraint (v5): dma_gather descriptors are generated by
GpSimd(Q7), which a DVE op in 2-port perf mode (tensor_scalar / copy / cast)
FULLY BLOCKS via the shared SBUF port — starving the gather pipeline. All
large elementwise work therefore uses only tensor_tensor /
scalar_tensor_tensor (1-port, never contend) with constant tiles, and the
Activation engine for copies; measured gather-only rate is ~3.96us per
1024x1KB gather vs ~5.2us with contending ops in flight.
"""
import numpy as np
import ml_dtypes

import concourse.bacc as bacc
import concourse.bass as bass
import concourse.tile as tile
import concourse.mybir as mybir
from concourse.bass_utils import run_bass_kernel_spmd
from concourse.masks import make_identity
from concourse.library_config import mlp

F32 = mybir.dt.float32
BF16 = mybir.dt.bfloat16
FP8 = mybir.dt.float8e4
I16 = mybir.dt.int16

V = 22528          # ft table rows
FT = 1024          # ft embedding dim
PSQT = 8           # psqt buckets (host-side now)
E = 1024           # gathered row length in fp8 bytes (256B-aligned)
STEP = 1280        # table row stride in bytes (non-pow2 to spread HBM banks)
B = 8192
FEATS = 32         # features per bag
NCORES = 8
BC = B // NCORES   # samples per core
P = 128            # partitions
T = BC // P        # sample tiles per core (8)
NI = 1024          # rows per dma_gather (FPG features x 128 samples)
FPG = NI // P      # features per gather (8)
GPB = FEATS // FPG  # gathers per bag per tile (4)
L2 = 15
MIN = mybir.AluOpType.min
MAX = mybir.AluOpType.max
MULT = mybir.AluOpType.mult
ADD = mybir.AluOpType.add

_CACHE = {}


def _build_nc(reps=1, nq=3, gbufs=12, scrmult=2):
    # SWDGE descriptor ring must hold one full gather (16B per descriptor);
    # scrmult>1 deepens the per-queue ring so desc-gen can run further ahead
    scratch = max(16384, NI * 16 * scrmult)
    nc = bacc.Bacc("TRN2", target_bir_lowering=False, debug=False,
                   num_swdge_queues=nq, dynamic_dma_scratch_size=scratch)

    tbl = nc.dram_tensor("tbl", [V, STEP], FP8, kind="ExternalInput")
    idx = nc.dram_tensor("idx", [P, T * 2 * GPB * (NI // 16)], I16,
                         kind="ExternalInput")
    m0 = nc.dram_tensor("m0", [P, T * 128], F32, kind="ExternalInput")
    m1 = nc.dram_tensor("m1", [P, T * 256], F32, kind="ExternalInput")
    m8 = nc.dram_tensor("m8", [P, T * 8], F32, kind="ExternalInput")
    psel_d = nc.dram_tensor("psel", [P, T], F32, kind="ExternalInput")
    cbias = nc.dram_tensor("cbias", [1, FT], F32, kind="ExternalInput")
    w0T = nc.dram_tensor("w0T", [P, FT], F32, kind="ExternalInput")
    w1T = nc.dram_tensor("w1T", [32, 256], F32, kind="ExternalInput")
    w2T = nc.dram_tensor("w2T", [32, 8], F32, kind="ExternalInput")
    b0 = nc.dram_tensor("b0", [1, 128], F32, kind="ExternalInput")
    b1 = nc.dram_tensor("b1", [1, 256], F32, kind="ExternalInput")
    b2 = nc.dram_tensor("b2", [1, 8], F32, kind="ExternalInput")
    out = nc.dram_tensor("out", [BC, 1], F32, kind="ExternalOutput")

    with tile.TileContext(nc) as tc:
        with tc.tile_pool(name="const", bufs=1) as cp, \
             tc.tile_pool(name="gat", bufs=gbufs) as gpool, \
             tc.tile_pool(name="accs", bufs=2) as apool, \
             tc.tile_pool(name="small", bufs=2) as spool, \
             tc.tile_pool(name="psum", bufs=2, space="PSUM") as pp, \
             tc.tile_pool(name="psred", bufs=2, space="PSUM") as pred, \
             tc.tile_pool(name="psacc", bufs=2, space="PSUM") as ppacc:

            nc.gpsimd.load_library(mlp)

            # ---- constants, loaded once ----
            idx_sb = cp.tile([P, T * 2 * GPB * (NI // 16)], I16)
            nc.sync.dma_start(idx_sb[:], idx[:])
            ident = cp.tile([P, P], F32)
            make_identity(nc, ident[:])
            id2 = cp.tile([P, 2 * P], FP8)
            nc.scalar.copy(id2[:, 0:P], ident[:])
            nc.scalar.copy(id2[:, P:2 * P], ident[:])
            cbias_sb = cp.tile([P, FT], F32)
            nc.sync.dma_start(cbias_sb[:], cbias[:].to_broadcast((P, FT)))
            w0T_sb = cp.tile([P, FT], F32)
            nc.sync.dma_start(w0T_sb[:], w0T[:])
            w1T_sb = cp.tile([32, 256], F32)
            nc.sync.dma_start(w1T_sb[:], w1T[:])
            w2T_sb = cp.tile([32, 8], F32)
            nc.sync.dma_start(w2T_sb[:], w2T[:])
            b0_sb = cp.tile([P, 128], F32)
            nc.sync.dma_start(b0_sb[:], b0[:].to_broadcast((P, 128)))
            b1_sb = cp.tile([P, 256], F32)
            nc.sync.dma_start(b1_sb[:], b1[:].to_broadcast((P, 256)))
            b2_sb = cp.tile([P, 8], F32)
            nc.sync.dma_start(b2_sb[:], b2[:].to_broadcast((P, 8)))
            # all-tile bucket one-hot masks + host-side psqt, loaded once
            m0_all = cp.tile([P, T * 128], F32)
            nc.sync.dma_start(m0_all[:], m0[:])
            m1_all = cp.tile([P, T * 256], F32)
            nc.sync.dma_start(m1_all[:], m1[:])
            m8_all = cp.tile([P, T * 8], F32)
            nc.sync.dma_start(m8_all[:], m8[:])
            psel_all = cp.tile([P, T], F32)
            nc.sync.dma_start(psel_all[:], psel_d[:])
            # constant tiles for tensor_tensor-class clips (these DVE op
            # classes never block GpSimd descriptor generation)
            c127 = cp.tile([P, 512], F32)
            nc.vector.memset(c127[:], 127.0)
            cinv64 = cp.tile([P, 32], F32)
            nc.vector.memset(cinv64[:], 1.0 / 64.0)
            cz = cp.tile([P, 2], F32)
            nc.vector.memset(cz[:], 0.0)

            for t in [t for _ in range(reps) for t in range(T)]:
                rows = slice(t * P, (t + 1) * P)
                m0_t = m0_all[:, t * 128:(t + 1) * 128]
                m1_t = m1_all[:, t * 256:(t + 1) * 256]
                m8_t = m8_all[:, t * 8:(t + 1) * 8]

                # ---- gather + reduce both bags ----
                accs = []
                for bag in range(2):
                    acc = apool.tile([P, FT], F32,
                                     tag="acc_w" if bag == 0 else "acc_b")
                    bufs = []
                    for g in range(GPB):
                        j = (t * 2 + bag) * GPB + g
                        col = j * (NI // 16)
                        buf = gpool.tile([P, FPG, E], FP8, tag="gather")
                        nc.gpsimd.dma_gather(
                            buf[:], tbl[:, 0:E], idx_sb[:, col:col + NI // 16],
                            NI, NI, E, elem_step=STEP,
                            transpose=False, queue_num=j % nq)
                        bufs.append(buf)

                    # PE: all gathers reduced via identity matmuls (fp8,
                    # DoubleRow), accumulated exactly in f32 PSUM
                    psA = pred.tile([P, 512], F32, tag="psA", space="PSUM")
                    psB = pred.tile([P, 512], F32, tag="psB", space="PSUM")
                    n_mm = GPB * FPG // 2
                    i_mm = 0
                    i2v = id2[:].rearrange("p (two f) -> p two f", two=2)
                    DR = mybir.MatmulPerfMode.DoubleRow
                    for g in range(GPB):
                        for flp in range(0, FPG, 2):
                            st = i_mm == 0
                            sp = i_mm == n_mm - 1
                            nc.tensor.matmul(
                                psA[:], lhsT=i2v,
                                rhs=bufs[g][:, flp:flp + 2, 0:512],
                                start=st, stop=sp, perf_mode=DR)
                            nc.tensor.matmul(
                                psB[:], lhsT=i2v,
                                rhs=bufs[g][:, flp:flp + 2, 512:1024],
                                start=st, stop=sp, perf_mode=DR)
                            i_mm += 1

                    # merge + exact f32 bias (tensor_tensor class)
                    nc.vector.tensor_add(acc[:, 0:512], psA[:],
                                         cbias_sb[:, 0:512])
                    nc.vector.tensor_add(acc[:, 512:1024], psB[:],
                                         cbias_sb[:, 512:1024])
                    accs.append(acc)
                acc_stm, acc_opp = accs

                # ---- pairwise: clip both halves, multiply, in place ----
                H = FT // 2
                ft_halves = []
                for acc in (acc_stm, acc_opp):
                    # clip(x,0,127) = (x max 0.0) min c127 — stt class
                    nc.vector.scalar_tensor_tensor(
                        acc[:, 0:H], acc[:, 0:H], 0.0, c127[:],
                        op0=MAX, op1=MIN)
                    nc.vector.scalar_tensor_tensor(
                        acc[:, H:FT], acc[:, H:FT], 0.0, c127[:],
                        op0=MAX, op1=MIN)
                    nc.vector.tensor_mul(acc[:, 0:H], acc[:, 0:H], acc[:, H:FT])
                    ft_halves.append(acc[:, 0:H])

                # ---- fc0: transpose ft tiles, matmul all stacks ----
                mm = ppacc.tile([P, 256], F32, tag="mm", space="PSUM")
                o0p = mm[:, 0:128]
                for k in range(8):
                    col = (k % 4) * P
                    tp = pp.tile([P, P], F32, tag="tpose", space="PSUM")
                    nc.tensor.transpose(tp[:], ft_halves[k // 4][:, col:col + P], ident[:])
                    ftT = spool.tile([P, P], F32, tag="ftT")
                    nc.scalar.copy(ftT[:], tp[:])
                    nc.tensor.matmul(
                        o0p, lhsT=ftT[:], rhs=w0T_sb[:, k * P:(k + 1) * P],
                        start=(k == 0), stop=(k == 7))

                # ---- fc0 bias + mask select ----
                o0m = spool.tile([P, 128], F32, tag="o0m")
                nc.vector.tensor_add(o0m[:], o0p, b0_sb[:])
                nc.vector.tensor_mul(o0m[:], o0m[:], m0_t)
                o0h = spool.tile([P, 64], F32, tag="o0h")
                nc.vector.tensor_add(o0h[:], o0m[:, 0:64], o0m[:, 64:128])
                nc.vector.tensor_add(o0h[:, 0:32], o0h[:, 0:32], o0h[:, 32:64])
                o0 = spool.tile([P, 16], F32, tag="o0")
                nc.vector.tensor_add(o0[:], o0h[:, 0:16], o0h[:, 16:32])

                # ---- slab activations (tt/stt classes + ACT copy only) ----
                slab = spool.tile([P, 32], F32, tag="slab")
                nc.scalar.copy(slab[:, 30:32], cz[:])
                sq = spool.tile([P, L2], F32, tag="sq")
                nc.vector.tensor_mul(sq[:], o0[:, 0:L2], o0[:, 0:L2])
                # sqr slab: (sq * 1/2^19) min 127; sq >= 0 so no lower clip
                nc.vector.scalar_tensor_tensor(
                    slab[:, 0:L2], sq[:], 1.0 / 524288.0, c127[:, 0:L2],
                    op0=MULT, op1=MIN)
                # rel slab: ((o0 max 0) * 1/64) min 127
                nc.vector.scalar_tensor_tensor(
                    slab[:, L2:2 * L2], o0[:, 0:L2], 0.0, cinv64[:, 0:L2],
                    op0=MAX, op1=MULT)
                nc.vector.tensor_tensor(
                    slab[:, L2:2 * L2], slab[:, L2:2 * L2], c127[:, 0:L2], op=MIN)

                # ---- fc1 ----
                tpf = pp.tile([P, P], F32, tag="tpose", space="PSUM")
                tps = tpf[0:32, :]
                nc.tensor.transpose(tps, slab[:], ident[:])
                slabT = spool.tile([32, P], F32, tag="slabT")
                nc.scalar.copy(slabT[:], tps)
                o1p = mm[:, 0:256]
                nc.tensor.matmul(o1p, lhsT=slabT[:], rhs=w1T_sb[:], start=True, stop=True)
                o1m = spool.tile([P, 256], F32, tag="o1m")
                nc.vector.tensor_add(o1m[:], o1p, b1_sb[:])
                nc.vector.tensor_mul(o1m[:], o1m[:], m1_t)
                o1h = spool.tile([P, 128], F32, tag="o1h")
                nc.vector.tensor_add(o1h[:], o1m[:, 0:128], o1m[:, 128:256])
                nc.vector.tensor_add(o1h[:, 0:64], o1h[:, 0:64], o1h[:, 64:128])
                o1 = spool.tile([P, 32], F32, tag="o1")
                nc.vector.tensor_add(o1[:], o1h[:, 0:32], o1h[:, 32:64])
                # ac1 = ((o1 max 0) * 1/64) min 127
                nc.vector.scalar_tensor_tensor(
                    o1[:], o1[:], 0.0, cinv64[:], op0=MAX, op1=MULT)
                nc.vector.tensor_tensor(o1[:], o1[:], c127[:, 0:32], op=MIN)

                # ---- fc2 ----
                tpg = pp.tile([P, P], F32, tag="tpose", space="PSUM")
                tpa = tpg[0:32, :]
                nc.tensor.transpose(tpa, o1[:], ident[:])
                ac1T = spool.tile([32, P], F32, tag="ac1T")
                nc.scalar.copy(ac1T[:], tpa)
                o2p = mm[:, 0:8]
                nc.tensor.matmul(o2p, lhsT=ac1T[:], rhs=w2T_sb[:], start=True, stop=True)
                o2m = spool.tile([P, 8], F32, tag="o2m")
                nc.vector.tensor_add(o2m[:], o2p, b2_sb[:])
                nc.vector.tensor_mul(o2m[:], o2m[:], m8_t)
                o2h = spool.tile([P, 4], F32, tag="o2h")
                nc.vector.tensor_add(o2h[:], o2m[:, 0:4], o2m[:, 4:8])
                nc.vector.tensor_add(o2h[:, 0:2], o2h[:, 0:2], o2h[:, 2:4])
                res = spool.tile([P, 1], F32, tag="res")
                nc.vector.tensor_add(res[:], o2h[:, 0:1], o2h[:, 1:2])

                # ---- skip + host-side psqt + output ----
                # res += o0[:,15] * (9600/8128/16)  (stt class)
                nc.vector.scalar_tensor_tensor(
                    res[:], o0[:, L2:16], 9600.0 / 8128.0 / 16.0, res[:],
                    op0=MULT, op1=ADD)
                nc.vector.tensor_add(res[:], res[:], psel_all[:, t:t + 1])
                nc.sync.dma_start(out[rows, :], res[:])

    nc.compile()
    return nc


def _prep_inputs(inputs):
    """Host-side prep: fp8 table, wrapped int16 gather indices (stm-swapped),
    transposed/prescaled weights, bucket one-hot masks, and the exact psqt
    term (tiny [V,8] table; computed here so the device gathers 1024B rows)."""
    ft_w = np.asarray(inputs["ft_w"], dtype=np.float32)
    psqt_w = np.asarray(inputs["psqt_w"], dtype=np.float32)
    ft_bias = np.asarray(inputs["ft_bias"], dtype=np.float32)
    tbl = np.zeros((V, STEP), dtype=ml_dtypes.float8_e4m3fn)
    tbl[:, :FT] = ft_w.astype(ml_dtypes.float8_e4m3fn)
    cbias = ft_bias.reshape(1, FT)

    fc0_w = np.asarray(inputs["fc0_w"], dtype=np.float32)  # [8,16,1024]
    fc1_w = np.asarray(inputs["fc1_w"], dtype=np.float32)  # [8,32,32]
    fc2_w = np.asarray(inputs["fc2_w"], dtype=np.float32)  # [8,1,32]
    a = fc0_w.transpose(2, 0, 1).reshape(FT, 128) * (1.0 / 128.0)   # [h, (s,j)]
    w0T = np.ascontiguousarray(
        a.reshape(8, 128, 128).transpose(1, 0, 2).reshape(128, FT))
    w1T = np.ascontiguousarray(fc1_w.transpose(2, 0, 1).reshape(32, 256))
    w2T = np.ascontiguousarray(fc2_w[:, 0, :].T * (1.0 / 16.0))     # [32, 8]
    b0 = np.asarray(inputs["fc0_b"], np.float32).reshape(1, 128)
    b1 = np.asarray(inputs["fc1_b"], np.float32).reshape(1, 256)
    b2 = np.asarray(inputs["fc2_b"], np.float32).reshape(1, 8) * (1.0 / 16.0)

    w_feats = np.asarray(inputs["w_feats"]).astype(np.int64)
    b_feats = np.asarray(inputs["b_feats"]).astype(np.int64)
    stm = np.asarray(inputs["stm"]).astype(np.float32)
    bucket = np.asarray(inputs["bucket"]).astype(np.int64)

    # exact host-side psqt: (psqt_stm - psqt_opp)[b, bucket] / 32
    # (the /2 from the reference plus the final /16 folded together)
    psqt_wb = psqt_w[w_feats.reshape(B, FEATS)].sum(axis=1)  # [B, 8]
    psqt_bb = psqt_w[b_feats.reshape(B, FEATS)].sum(axis=1)
    stm_col = stm.reshape(B, 1)
    psqt_stm = psqt_wb * (1 - stm_col) + psqt_bb * stm_col
    psqt_opp = psqt_bb * (1 - stm_col) + psqt_wb * stm_col
    bidx = np.arange(B)
    psel = (psqt_stm[bidx, bucket] - psqt_opp[bidx, bucket]) / 32.0  # [B]

    w_feats16 = w_feats.astype(np.int16)
    b_feats16 = b_feats.astype(np.int16)

    in_maps = []
    for c in range(NCORES):
        s = slice(c * BC, (c + 1) * BC)
        wf = w_feats16[c * BC * FEATS:(c + 1) * BC * FEATS].reshape(T, P, FEATS)
        bf = b_feats16[c * BC * FEATS:(c + 1) * BC * FEATS].reshape(T, P, FEATS)
        st_tp = stm[s].reshape(T, P, 1) > 0.5
        f_stm = np.where(st_tp, bf, wf)
        f_opp = np.where(st_tp, wf, bf)
        # gather j=(t*2+bag)*GPB+g covers features FPG*g..FPG*(g+1)-1,
        # feature-major: local id i = f_local*128 + sample
        blocks = np.empty((T, 2, GPB, NI), np.int16)
        for bag, arr in enumerate((f_stm, f_opp)):
            ak = arr.reshape(T, P, GPB, FPG).transpose(0, 2, 3, 1)  # [T,GPB,FPG,P]
            blocks[:, bag, :, :] = ak.reshape(T, GPB, NI)
        flat = blocks.reshape(T * 2 * GPB, NI)
        wrapped = flat.reshape(-1, NI // 16, 16).transpose(0, 2, 1)
        idx_arr = np.zeros((P, T * 2 * GPB * (NI // 16)), np.int16)
        cols = wrapped.transpose(1, 0, 2).reshape(16, -1)
        for g in range(8):
            idx_arr[g * 16:(g + 1) * 16, :] = cols

        bk = bucket[s]
        m0 = (bk[:, None] == (np.arange(128) // 16)).astype(np.float32)
        m1 = (bk[:, None] == (np.arange(256) // 32)).astype(np.float32)
        m8 = (bk[:, None] == np.arange(8)).astype(np.float32)
        # SBUF layout: partition = sample-in-tile, cols = tile-major blocks
        m0 = np.ascontiguousarray(
            m0.reshape(T, P, 128).transpose(1, 0, 2).reshape(P, T * 128))
        m1 = np.ascontiguousarray(
            m1.reshape(T, P, 256).transpose(1, 0, 2).reshape(P, T * 256))
        m8 = np.ascontiguousarray(
            m8.reshape(T, P, 8).transpose(1, 0, 2).reshape(P, T * 8))
        psel_c = np.ascontiguousarray(
            psel[s].astype(np.float32).reshape(T, P).T)  # [P, T]
        in_maps.append({
            "tbl": tbl, "idx": idx_arr, "cbias": cbias,
            "m0": m0, "m1": m1, "m8": m8, "psel": psel_c,
            "w0T": w0T, "w1T": w1T, "w2T": w2T,
            "b0": b0, "b1": b1, "b2": b2,
        })
    return in_maps


def kernel(**inputs) -> np.ndarray:
    if "nc" not in _CACHE:
        _CACHE["nc"] = _build_nc()
    nc = _CACHE["nc"]
    in_maps = _prep_inputs(inputs)
    r = run_bass_kernel_spmd(nc, in_maps, core_ids=list(range(NCORES)))
    return np.concatenate([r.results[c]["out"][:, 0] for c in range(NCORES)])
